# revision 16
# baseline (speedup 1.0000x reference)
"""Trainium2 Bass kernel for nn_Encoder_73778948211333.

6-layer transformer encoder (no qkv projections: q=k=v=head slices of x),
B=4, S=2048, D=512, H=8 heads, DFF=2048, fp32, no activation between fc1/fc2.

Sharding: 8 cores = (batch, sequence-half). Each core owns 1024 query rows of
one batch: computes attention for its rows (k-major scores -> exp -> PV with a
fused ones-column rowsum), wo + LN1 + FFN + LN2 for its rows, then a PAIRWISE
AllGather ([0,1],[2,3],...) exchanges updated halves between layers. The
payload carries the already-transposed feature-major tiles (xT, fp16) and the
token-major value tiles (xtok, bf16) so the receiver does zero rebuild
compute — peer tiles land via indirect row-gather DMAs.

All matmuls run in fp16/bf16 (full PE rate with fast-weight-load; the fp32
"HIGH" mode the previous version used streams at less than half rate on real
silicon). Softmax skips max-subtraction: scores are bounded since every layer
output is layer-normalized; exp outputs bf16 (large dynamic range).
LN gains/biases that are exactly 1/0 in the inputs are skipped at build time
(checked host-side; a general build is used otherwise).
"""

import sys

sys.path.insert(0, "/opt/trn_rl_repo")
sys.path.insert(0, "/root/.axon_site")

import numpy as np
import ml_dtypes

import concourse.bass as bass
import concourse.tile as tile
from concourse import bacc, mybir
from concourse.bass import ds, ts
from concourse.masks import make_identity

# ---- problem constants (hardcoded per spec) ----
B, S, D = 4, 2048, 512
H, DK = 8, 64
DFF = 4 * D
N_LAYERS = 6
EPS = 1e-8
P = 128
NC = 8
QH = S // 2          # 1024 rows per core
NKB = S // P         # 16 k-blocks
NQT = QH // P        # 8 q-tiles per core
NQC = QH // 512      # 2 q-chunks of 512
XTW = H * (DK + 1)   # 520: token-major row width incl. ones columns
HS = DK + 1          # per-head stride in xtok

F32 = mybir.dt.float32
F16 = mybir.dt.float16
BF16 = mybir.dt.bfloat16
I32 = mybir.dt.int32
AF = mybir.ActivationFunctionType
ALU = mybir.AluOpType


def build(n_layers=N_LAYERS, use_affine=False):
    nc = bacc.Bacc("TRN2", target_bir_lowering=False, debug=False, num_devices=NC)

    # ---- I/O ----
    x_feat = nc.declare_dram_parameter("x_feat", [D, S], F16, isOutput=False)
    x_tok = nc.declare_dram_parameter("x_tok", [S, XTW], BF16, isOutput=False)
    x_own = nc.declare_dram_parameter("x_own", [QH, D], F16, isOutput=False)
    idxf_in = nc.declare_dram_parameter("idxf", [P, D // P], I32, isOutput=False)
    idxt_in = nc.declare_dram_parameter("idxt", [P, NQT], I32, isOutput=False)
    # woT blocked [l, oc, 128, D]: rhs tiles for the wo matmul
    woT_d = nc.declare_dram_parameter("woT", [n_layers, D // P, P, D], F16, isOutput=False)
    # w1T blocked [l, jc, 128(p=d), 4(oc), 128(j)]: one [128,512] tile per jc
    w1T_d = nc.declare_dram_parameter(
        "w1T", [n_layers, DFF // P, P, D // P, P], F16, isOutput=False)
    # w2T blocked [l, jc, 128(p=j), D(o)]
    w2T_d = nc.declare_dram_parameter("w2T", [n_layers, DFF // P, P, D], F16, isOutput=False)
    wob_d = nc.declare_dram_parameter("wob", [n_layers, D], F32, isOutput=False)
    fc1b_d = nc.declare_dram_parameter("fc1b", [n_layers, DFF], F32, isOutput=False)
    fc2b_d = nc.declare_dram_parameter("fc2b", [n_layers, D], F32, isOutput=False)
    g1_d = nc.declare_dram_parameter("g1", [n_layers, D], F32, isOutput=False)
    b1_d = nc.declare_dram_parameter("b1", [n_layers, D], F32, isOutput=False)
    g2_d = nc.declare_dram_parameter("g2", [n_layers, D], F32, isOutput=False)
    b2_d = nc.declare_dram_parameter("b2", [n_layers, D], F32, isOutput=False)
    out_d = nc.declare_dram_parameter("out", [QH, D], F16, isOutput=True)

    # collective staging: xT own half [4, 128, 1024] f16 + xtok own [8, 128, 520] bf16
    agf_ins = [nc.dram_tensor(f"agf_in{l}", [D // P, P, QH], F16) for l in range(n_layers - 1)]
    agt_ins = [nc.dram_tensor(f"agt_in{l}", [NQT, P, XTW], BF16) for l in range(n_layers - 1)]
    agf_outs = [
        nc.dram_tensor(f"agf_out{l}", [2, D // P, P, QH], F16)
        for l in range(n_layers - 1)
    ]
    agt_outs = [
        nc.dram_tensor(f"agt_out{l}", [2, NQT, P, XTW], BF16)
        for l in range(n_layers - 1)
    ]

    with nc.allow_low_precision(reason="deliberate f16/bf16 compute"), tile.TileContext(nc) as tc:
        from contextlib import ExitStack

        with ExitStack() as ctx:
            persist = ctx.enter_context(tc.tile_pool(name="persist", bufs=1))
            wo_pool = ctx.enter_context(tc.tile_pool(name="wo_pool", bufs=5))
            w1_pool = ctx.enter_context(tc.tile_pool(name="w1_pool", bufs=4))
            w2_pool = ctx.enter_context(tc.tile_pool(name="w2_pool", bufs=16))
            b_pool = ctx.enter_context(tc.tile_pool(name="b_pool", bufs=7))
            fb_pool = ctx.enter_context(tc.tile_pool(name="fb_pool", bufs=2))
            e_pool = ctx.enter_context(tc.tile_pool(name="e_pool", bufs=3))
            h_pool = ctx.enter_context(tc.tile_pool(name="h_pool", bufs=16))
            work = ctx.enter_context(tc.tile_pool(name="work", bufs=4))
            small = ctx.enter_context(tc.tile_pool(name="small", bufs=6))
            op_pool = ctx.enter_context(tc.tile_pool(name="op_pool", bufs=8))
            # PSUM budget (16KB/partition): duo 2x4KB + facc 1x4KB + pv 2x2KB
            ps = ctx.enter_context(tc.tile_pool(name="ps", bufs=2, space="PSUM"))

            # ---- persistent state ----
            xT = [persist.tile([P, S], F16, tag=f"xT{i}", name=f"xT{i}") for i in range(D // P)]
            xtok = [persist.tile([P, XTW], BF16, tag=f"xtok{i}", name=f"xtok{i}") for i in range(NKB)]
            xres = [persist.tile([P, D], F16, tag=f"xres{i}", name=f"xres{i}") for i in range(NQT)]
            x1 = [persist.tile([P, D], F16, tag=f"x1_{i}", name=f"x1_{i}") for i in range(NQT)]
            x1T = [persist.tile([P, QH], F16, tag=f"x1T{i}", name=f"x1T{i}") for i in range(D // P)]
            oT = [persist.tile([P, QH], F16, tag=f"oT{i}", name=f"oT{i}") for i in range(D // P)]
            identh = persist.tile([P, P], F16, tag="identh")
            identf = persist.tile([P, P], F32, tag="identf")
            ones1h = persist.tile([1, DK], F16, tag="ones1h")
            epsT = persist.tile([P, 1], F32, tag="epsT")
            # partial softmax denominators: one [1, 1024] tile per (qc, pair)
            # group (hp0 in cols 0:512, hp1 in 512:1024; partition 0 only —
            # partition bases must be 32-aligned so no [2, ...] row tiles)
            sparts = [persist.tile([1, 1024], F32, tag=f"sp{g}", name=f"sp{g}")
                      for g in range(NQC * 4)]
            idxf_sb = persist.tile([P, D // P], I32, tag="idxf_sb")
            idxt_sb = persist.tile([P, NQT], I32, tag="idxt_sb")

            make_identity(nc, identf[:])
            nc.vector.tensor_copy(out=identh[:], in_=identf[:])
            nc.vector.memset(ones1h[:], 1.0)
            nc.vector.memset(epsT[:], EPS)
            nc.sync.dma_start(idxf_sb[:], idxf_in[:])
            nc.sync.dma_start(idxt_sb[:], idxt_in[:])

            # ---- initial loads ----
            for i in range(D // P):
                nc.sync.dma_start(xT[i][:], x_feat[ts(i, P), :])
            for i in range(NKB):
                nc.sync.dma_start(xtok[i][:], x_tok[ts(i, P), :])
            for i in range(NQT):
                nc.sync.dma_start(xres[i][:], x_own[ts(i, P), :])

            def load_layer_weights(l):
                woT_sb = []
                for oc in range(D // P):
                    t = wo_pool.tile([P, D], F16, tag="woT_sb", name="woT_sb")
                    nc.sync.dma_start(t[:], woT_d[l, oc])
                    woT_sb.append(t)
                bc = {}
                if use_affine:
                    for name, dram in (
                        ("wob", wob_d), ("fc2b", fc2b_d),
                        ("g1", g1_d), ("b1", b1_d), ("g2", g2_d), ("b2", b2_d),
                    ):
                        t = b_pool.tile([P, D], F32, tag="bc", name="bc")
                        nc.sync.dma_start(t[:], dram[l, None, :].to_broadcast((P, D)))
                        bc[name] = t
                    fc1b_sb = fb_pool.tile([P, DFF // P], F32, tag="fc1b_sb")
                    nc.sync.dma_start(
                        fc1b_sb[:], fc1b_d[l, :].rearrange("(a p) -> p a", p=P)
                    )
                else:
                    fc1b_sb = None
                return woT_sb, bc, fc1b_sb

            def recip_fast(out_ap, in_ap):
                from concourse.dve_ops import (
                    RECIP_APPROX_FAST_CONSTS,
                    RECIPROCAL_APPROX_FAST,
                )

                c = RECIP_APPROX_FAST_CONSTS
                nc.vector._custom_dve(
                    RECIPROCAL_APPROX_FAST, out=out_ap, in0=in_ap,
                    s0=c["s0"], s1=c["s1"], imm2=c["imm2"],
                )

            def attn_kb(pair, qc, pv, kb, start, stop):
                """One k-block: 2 scores MMs + exp + 2 PV MMs."""
                q0 = qc * 512
                duo = ps.tile([P, 1024], F32, tag="duo", name="duo")
                for hp in range(2):
                    nc.tensor.matmul(
                        duo[:, ts(hp, 512)],
                        xT[pair][ts(hp, DK), ts(kb, P)],
                        xT[pair][ts(hp, DK), ds(q0, 512)],
                        start=True, stop=True,
                    )
                e_t = e_pool.tile([P, 1024], BF16, tag="e", name="e_t")
                nc.scalar.activation(e_t[:], duo[:], AF.Exp, scale=1.0 / np.sqrt(DK))
                for hp in range(2):
                    h = 2 * pair + hp
                    nc.tensor.matmul(
                        pv[hp][0 : DK + 1, :],
                        xtok[kb][:, ds(h * HS, DK + 1)], e_t[:, ts(hp, 512)],
                        start=start, stop=stop,
                    )

            def attention_part1(l, pair, qc):
                """Own-half k-blocks; evicts partial O (bf16 SBUF) and the
                partial rowsum rows (SBUF scratch), freeing PSUM while the
                AllGather completes in the background."""
                pv = [ps.tile([P, 512], F32, tag="pv", name="pv") for _ in range(2)]
                for kb in range(NKB // 2):
                    attn_kb(pair, qc, pv, kb, kb == 0, kb == NKB // 2 - 1)
                g = qc * 4 + pair
                opart = op_pool.tile([P, 512], BF16, tag="opart", name="opart")
                for hp in range(2):
                    nc.vector.tensor_copy(
                        out=opart[ts(hp, DK), :], in_=pv[hp][0:DK, :]
                    )
                    nc.vector.tensor_copy(
                        out=sparts[g][0:1, ts(hp, 512)], in_=pv[hp][DK : DK + 1, :]
                    )
                return opart

            def part2_epilogue(pair, qc, pv, opart):
                """Combine peer-half PV with part1 + normalize into oT."""
                q0 = qc * 512
                g = qc * 4 + pair
                s2 = small.tile([1, 1024], F32, tag="s_sb", name="s2")
                for hp in range(2):
                    nc.vector.tensor_add(
                        s2[0:1, ts(hp, 512)], pv[hp][DK : DK + 1, :],
                        sparts[g][0:1, ts(hp, 512)],
                    )
                rf = small.tile([1, 1024], F32, tag="s_sb", name="rf")
                recip_fast(rf[:], s2[:])
                r2 = small.tile([1, 1024], F16, tag="r2", name="r2")
                nc.vector.tensor_copy(out=r2[:], in_=rf[:])
                bc_ps = ps.tile([P, 512], F32, tag="pv", name="bc_ps")
                for hp in range(2):
                    nc.tensor.matmul(
                        bc_ps[ts(hp, DK), :], ones1h[:], r2[0:1, ts(hp, 512)],
                        start=True, stop=True,
                    )
                for hp in range(2):
                    o_un = work.tile([P, 512], F32, tag="work", name="o_un")
                    nc.vector.tensor_add(
                        o_un[0:DK, :], pv[hp][0:DK, :], opart[ts(hp, DK), :]
                    )
                    dst = oT[pair][ts(hp, DK), ds(q0, 512)]
                    nc.vector.tensor_mul(dst, o_un[0:DK, :], bc_ps[ts(hp, DK), :])

            def ln_stats(src_tile):
                stats = small.tile([P, 6], F32, tag="stats")
                nc.vector.bn_stats(out=stats[:], in_=src_tile[:])
                mv = small.tile([P, 2], F32, tag="mv")
                nc.vector.bn_aggr(out=mv[:], in_=stats[:])
                return mv

            def ln_rstd_batch(mvs):
                n = len(mvs)
                vb = small.tile([P, 8], F32, tag="vb", name="vb")
                for i, mv in enumerate(mvs):
                    nc.vector.tensor_copy(out=vb[:, i : i + 1], in_=mv[:, 1:2])
                nc.scalar.activation(
                    out=vb[:, :n], in_=vb[:, :n], func=AF.Sqrt,
                    bias=epsT[:], scale=1.0,
                )
                nc.vector.reciprocal_approx_fast(out=vb[:, :n], in_=vb[:, :n])
                return vb

            def ln_apply(dst, src_tile, mv, rstd1, g_bc, b_bc):
                nc.vector.tensor_scalar(
                    out=dst[:], in0=src_tile[:],
                    scalar1=mv[:, 0:1], scalar2=rstd1,
                    op0=ALU.subtract, op1=ALU.mult,
                )
                if use_affine:
                    nc.vector.tensor_mul(dst[:], dst[:], g_bc[:])
                    nc.vector.tensor_add(dst[:], dst[:], b_bc[:])

            def wo_ln1(l, half, woT_sb, bc):
                ts_, mvs = [], []
                for q4 in range(4):
                    qt = half * 4 + q4
                    y_ps = ps.tile([P, D], F32, tag="duo", name="y_ps")
                    for oc in range(D // P):
                        nc.tensor.matmul(
                            y_ps[:], oT[oc][:, ts(qt, P)], woT_sb[oc][:],
                            start=(oc == 0), stop=(oc == D // P - 1),
                        )
                    t = work.tile([P, D], F32, tag="work")
                    nc.vector.tensor_add(t[:], y_ps[:], xres[qt][:])
                    if use_affine:
                        nc.vector.tensor_add(t[:], t[:], bc["wob"][:])
                    ts_.append(t)
                    mvs.append(ln_stats(t))
                vb = ln_rstd_batch(mvs)
                for q4 in range(4):
                    qt = half * 4 + q4
                    ln_apply(x1[qt], ts_[q4], mvs[q4], vb[:, q4 : q4 + 1],
                             bc.get("g1"), bc.get("b1"))
                    for ft in range(D // P):
                        tp = ps.tile([P, P], F16, tag="duo", name="tp")
                        nc.tensor.transpose(tp[:], x1[qt][:, ts(ft, P)], identh[:])
                        nc.vector.tensor_copy(out=x1T[ft][:, ts(qt, P)], in_=tp[:])

            def ffn_s1_step(l, qc, jc, facc01, hTs, w2s, fc1b_sb):
                """fc1(jc) + hT store + fc2 into q-tiles 0,1."""
                h_ps = ps.tile([P, 512], F32, tag="duo", name="h_ps")
                w1c = w1_pool.tile([P, D], F16, tag="w1c", name="w1c")
                nc.sync.dma_start(
                    w1c[:], w1T_d[l, jc].rearrange("p a j -> p (a j)")
                )
                for oc in range(D // P):
                    nc.tensor.matmul(
                        h_ps[:], w1c[:, ts(oc, P)],
                        x1T[oc][:, ds(qc * 512, 512)],
                        start=(oc == 0), stop=(oc == D // P - 1),
                    )
                hT = h_pool.tile([P, 512], F16, tag="hT", name="hT")
                if use_affine:
                    nc.vector.tensor_scalar_add(hT[:], h_ps[:], fc1b_sb[:, jc : jc + 1])
                else:
                    nc.vector.tensor_copy(out=hT[:], in_=h_ps[:])
                hTs.append(hT)
                w2c = w2_pool.tile([P, D], F16, tag="w2c", name="w2c")
                nc.sync.dma_start(w2c[:], w2T_d[l, jc])
                w2s.append(w2c)
                for q4 in range(2):
                    nc.tensor.matmul(
                        facc01[:, ts(q4, 512)], hTs[jc][:, ts(q4, P)], w2s[jc][:],
                        start=(jc == 0), stop=(jc == DFF // P - 1),
                    )

            def ffn_s2_step(l, qc, jc, facc23, hTs, w2s):
                for q4 in range(2, 4):
                    nc.tensor.matmul(
                        facc23[:, ts(q4 - 2, 512)], hTs[jc][:, ts(q4, P)], w2s[jc][:],
                        start=(jc == 0), stop=(jc == DFF // P - 1),
                    )

            def ffn_t2(qc, q4, facc_half, bc):
                qt = qc * 4 + q4
                t2 = work.tile([P, D], F32, tag="work")
                nc.vector.tensor_add(t2[:], facc_half[:, ts(q4 % 2, 512)], x1[qt][:])
                if use_affine:
                    nc.vector.tensor_add(t2[:], t2[:], bc["fc2b"][:])
                return t2, ln_stats(t2)

            def ffn_finish(l, qc, t2s, mvs2, bc):
                vb2 = ln_rstd_batch(mvs2)
                for q4 in range(4):
                    qt = qc * 4 + q4
                    ln_apply(xres[qt], t2s[q4], mvs2[q4],
                             vb2[:, q4 : q4 + 1], bc.get("g2"), bc.get("b2"))
                    if l == n_layers - 1:
                        nc.sync.dma_start(out_d[ts(qt, P), :], xres[qt][:])

            def rebuild_qt(l, i):
                """xtok tile i and xT col-block i from xres[i] (= new x)."""
                src3 = xres[i][:].rearrange("p (h k) -> p h k", k=DK)
                dst3 = xtok[i][:].rearrange("p (h k) -> p h k", k=HS)
                nc.vector.tensor_copy(out=dst3[:, :, 0:DK], in_=src3)
                nc.sync.dma_start(agt_ins[l][i], xtok[i][:])
                for ft in range(D // P):
                    tp = ps.tile([P, P], F16, tag="duo", name="tp")
                    nc.tensor.transpose(tp[:], xres[i][:, ts(ft, P)], identh[:])
                    nc.vector.tensor_copy(out=xT[ft][:, ts(i, P)], in_=tp[:])

            def fetch_peer(l):
                """xT cols 1024:2048 / xtok tiles 8..15 straight from the
                AllGather output — no rebuild compute on the receiver."""
                agf_flat = agf_outs[l][:].rearrange("r f p q -> (r f p) q")
                for ft in range(D // P):
                    nc.gpsimd.indirect_dma_start(
                        out=xT[ft][:, QH:S],
                        out_offset=None,
                        in_=agf_flat,
                        in_offset=bass.IndirectOffsetOnAxis(
                            ap=idxf_sb[:, ft : ft + 1], axis=0
                        ),
                    )
                agt_flat = agt_outs[l][:].rearrange("r i p w -> (r i p) w")
                for i in range(NQT):
                    nc.gpsimd.indirect_dma_start(
                        out=xtok[NQT + i][:],
                        out_offset=None,
                        in_=agt_flat,
                        in_offset=bass.IndirectOffsetOnAxis(
                            ap=idxt_sb[:, i : i + 1], axis=0
                        ),
                    )

            def emit_part1(l):
                oparts = {}
                for qc in range(NQC):
                    for pair in range(4):
                        oparts[(qc, pair)] = attention_part1(l, pair, qc)
                return oparts

            def part2_alone(l, qc, oparts):
                for pair in range(4):
                    pv = [ps.tile([P, 512], F32, tag="pv", name="pv") for _ in range(2)]
                    for kb_i in range(NKB // 2):
                        attn_kb(pair, qc, pv, NKB // 2 + kb_i,
                                kb_i == 0, kb_i == NKB // 2 - 1)
                    part2_epilogue(pair, qc, pv, oparts[(qc, pair)])

            def ffn_sweeps(l, qc, bc, fc1b_sb, attn=None, rebuild=False):
                """Two-sweep FFN for q-chunk qc. If `attn` is (qc_a, oparts),
                part2(qc_a) k-block steps interleave 2:1 with sweep-1 jc
                steps so softmax exps keep the scalar engine fed while the
                PE runs FFN matmuls. If `rebuild`, rebuild_qt(0..3) steps
                interleave instead (their xres came from the prior qc)."""
                facc01 = ps.tile([P, 1024], F32, tag="facc", name="facc01", bufs=1)
                hTs, w2s = [], []
                jc = 0
                if attn is not None:
                    qc_a, oparts = attn
                    for pair in range(4):
                        pv = [ps.tile([P, 512], F32, tag="pv", name="pv") for _ in range(2)]
                        for kb_i in range(NKB // 2):
                            attn_kb(pair, qc_a, pv, NKB // 2 + kb_i,
                                    kb_i == 0, kb_i == NKB // 2 - 1)
                            if kb_i % 2 == 1:
                                ffn_s1_step(l, qc, jc, facc01, hTs, w2s, fc1b_sb)
                                jc += 1
                        part2_epilogue(pair, qc_a, pv, oparts[(qc_a, pair)])
                while jc < DFF // P:
                    ffn_s1_step(l, qc, jc, facc01, hTs, w2s, fc1b_sb)
                    if rebuild and jc % 4 == 3 and l < n_layers - 1:
                        rebuild_qt(l, jc // 4)
                    jc += 1
                t2s, mvs2 = [], []
                for q4 in range(2):
                    t2, mv = ffn_t2(qc, q4, facc01, bc)
                    t2s.append(t2); mvs2.append(mv)
                facc23 = ps.tile([P, 1024], F32, tag="facc", name="facc23", bufs=1)
                for jc in range(DFF // P):
                    ffn_s2_step(l, qc, jc, facc23, hTs, w2s)
                for q4 in range(2, 4):
                    t2, mv = ffn_t2(qc, q4, facc23, bc)
                    t2s.append(t2); mvs2.append(mv)
                ffn_finish(l, qc, t2s, mvs2, bc)

            # ---- the stack ----
            oparts = None
            for l in range(n_layers):
                woT_sb, bc, fc1b_sb = load_layer_weights(l)
                if l == 0:
                    oparts = emit_part1(0)
                part2_alone(l, 0, oparts)
                wo_ln1(l, 0, woT_sb, bc)
                ffn_sweeps(l, 0, bc, fc1b_sb, attn=(1, oparts))
                wo_ln1(l, 1, woT_sb, bc)
                ffn_sweeps(l, 1, bc, fc1b_sb, rebuild=True)
                if l < n_layers - 1:
                    for i in range(4, NQT):
                        rebuild_qt(l, i)
                    for ft in range(D // P):
                        nc.sync.dma_start(agf_ins[l][ft], xT[ft][:, 0:QH])
                    nc.gpsimd.collective_compute(
                        "AllGather", ALU.bypass,
                        ins=[agf_ins[l][:].opt()],
                        outs=[agf_outs[l][:].opt()],
                        replica_groups=[[0, 1], [2, 3], [4, 5], [6, 7]],
                    )
                    nc.gpsimd.collective_compute(
                        "AllGather", ALU.bypass,
                        ins=[agt_ins[l][:].opt()],
                        outs=[agt_outs[l][:].opt()],
                        replica_groups=[[0, 1], [2, 3], [4, 5], [6, 7]],
                    )
                    oparts = emit_part1(l + 1)
                    fetch_peer(l)

    nc.compile()
    return nc


# ---- host side ----

_cache = {}


def _get_nc(n_layers=N_LAYERS, use_affine=False):
    key = (n_layers, use_affine)
    if key not in _cache:
        _cache[key] = build(n_layers, use_affine)
    return _cache[key]


def _trivial_affine(inputs, n_layers):
    return (
        not np.any(np.asarray(inputs["wo_b"], np.float32)[:n_layers])
        and not np.any(np.asarray(inputs["fc1_b"], np.float32)[:n_layers])
        and not np.any(np.asarray(inputs["fc2_b"], np.float32)[:n_layers])
        and not np.any(np.asarray(inputs["ln1_b"], np.float32)[:n_layers])
        and not np.any(np.asarray(inputs["ln2_b"], np.float32)[:n_layers])
        and np.all(np.asarray(inputs["ln1_g"], np.float32)[:n_layers] == 1.0)
        and np.all(np.asarray(inputs["ln2_g"], np.float32)[:n_layers] == 1.0)
    )


def make_in_maps(inputs, n_layers=N_LAYERS):
    f16 = ml_dtypes.float16 if hasattr(ml_dtypes, "float16") else np.float16
    x = np.asarray(inputs["x"], dtype=np.float32)
    woT = np.asarray(inputs["wo_w"], np.float32)[:n_layers].transpose(0, 2, 1)
    woT = np.ascontiguousarray(
        woT.reshape(n_layers, D // P, P, D)).astype(np.float16)
    w1T = np.asarray(inputs["fc1_w"], np.float32)[:n_layers].transpose(0, 2, 1)
    # [l, d, j] -> [l, jc, p(d-part within oc? no: p is d%?)]
    # desired tile[l, jc, p, oc, jj] = w1T[l, oc*128+p, jc*128+jj]
    w1T = w1T.reshape(n_layers, D // P, P, DFF // P, P).transpose(0, 3, 2, 1, 4)
    w1T = np.ascontiguousarray(w1T).astype(np.float16)
    w2T = np.asarray(inputs["fc2_w"], np.float32)[:n_layers].transpose(0, 2, 1)
    w2T = np.ascontiguousarray(
        w2T.reshape(n_layers, DFF // P, P, D)).astype(np.float16)
    common = {
        "woT": woT, "w1T": w1T, "w2T": w2T,
        "wob": np.ascontiguousarray(np.asarray(inputs["wo_b"], np.float32)[:n_layers]),
        "fc1b": np.ascontiguousarray(np.asarray(inputs["fc1_b"], np.float32)[:n_layers]),
        "fc2b": np.ascontiguousarray(np.asarray(inputs["fc2_b"], np.float32)[:n_layers]),
        "g1": np.ascontiguousarray(np.asarray(inputs["ln1_g"], np.float32)[:n_layers]),
        "b1": np.ascontiguousarray(np.asarray(inputs["ln1_b"], np.float32)[:n_layers]),
        "g2": np.ascontiguousarray(np.asarray(inputs["ln2_g"], np.float32)[:n_layers]),
        "b2": np.ascontiguousarray(np.asarray(inputs["ln2_b"], np.float32)[:n_layers]),
    }
    in_maps = []
    for c in range(NC):
        b, half = c // 2, c % 2
        own = x[b, half * QH : (half + 1) * QH]        # [QH, D]
        peer = x[b, (1 - half) * QH : (2 - half) * QH]
        local = np.concatenate([own, peer], axis=0)     # [S, D] core-relative
        x_feat = np.ascontiguousarray(local.T).astype(np.float16)  # [D, S]
        xt = np.zeros((S, H, HS), np.float32)
        xt[:, :, :DK] = local.reshape(S, H, DK)
        xt[:, :, DK] = 1.0
        x_tok = xt.reshape(S, XTW).astype(ml_dtypes.bfloat16)
        # peer slot within the 2-rank AllGather group
        pr = 1 - half
        p_ar = np.arange(P, dtype=np.int32)
        idxf = np.ascontiguousarray(
            (pr * (D // P) * P + np.arange(D // P, dtype=np.int32)[None, :] * P
             + p_ar[:, None]))
        idxt = np.ascontiguousarray(
            (pr * NQT * P + np.arange(NQT, dtype=np.int32)[None, :] * P
             + p_ar[:, None]))
        m = dict(common)
        m.update({
            "x_feat": x_feat, "x_tok": x_tok,
            "x_own": np.ascontiguousarray(own).astype(np.float16),
            "idxf": idxf, "idxt": idxt,
        })
        in_maps.append(m)
    return in_maps


def assemble_output(results):
    out = np.empty((B, S, D), np.float32)
    for c in range(NC):
        b, half = c // 2, c % 2
        out[b, half * QH : (half + 1) * QH] = np.asarray(
            results[c]["out"], dtype=np.float32)
    return out


def kernel(**inputs):
    from concourse.bass_utils import run_bass_kernel_spmd

    use_affine = not _trivial_affine(inputs, N_LAYERS)
    nc = _get_nc(N_LAYERS, use_affine)
    in_maps = make_in_maps(inputs)
    res = run_bass_kernel_spmd(nc, in_maps, core_ids=list(range(NC)))
    return assemble_output(res.results)


# revision 17
# speedup vs baseline: 1.0192x; 1.0192x over previous
"""Trainium2 Bass kernel for nn_Encoder_73778948211333.

6-layer transformer encoder (no qkv projections: q=k=v=head slices of x),
B=4, S=2048, D=512, H=8 heads, DFF=2048, fp32, no activation between fc1/fc2.

Sharding: 8 cores = (batch, sequence-half). Each core owns 1024 query rows of
one batch: computes attention for its rows (k-major scores -> exp -> PV with a
fused ones-column rowsum), wo + LN1 + FFN + LN2 for its rows, then a PAIRWISE
AllGather ([0,1],[2,3],...) exchanges updated halves between layers. The
payload carries the already-transposed feature-major tiles (xT, fp16) and the
token-major value tiles (xtok, bf16) so the receiver does zero rebuild
compute — peer tiles land via indirect row-gather DMAs.

All matmuls run in fp16/bf16 (full PE rate with fast-weight-load; the fp32
"HIGH" mode the previous version used streams at less than half rate on real
silicon). Softmax skips max-subtraction: scores are bounded since every layer
output is layer-normalized; exp outputs bf16 (large dynamic range).
LN gains/biases that are exactly 1/0 in the inputs are skipped at build time
(checked host-side; a general build is used otherwise).
"""

import sys

sys.path.insert(0, "/opt/trn_rl_repo")
sys.path.insert(0, "/root/.axon_site")

import numpy as np
import ml_dtypes

import concourse.bass as bass
import concourse.tile as tile
from concourse import bacc, mybir
from concourse.bass import ds, ts
from concourse.masks import make_identity

# ---- problem constants (hardcoded per spec) ----
B, S, D = 4, 2048, 512
H, DK = 8, 64
DFF = 4 * D
N_LAYERS = 6
EPS = 1e-8
P = 128
NC = 8
QH = S // 2          # 1024 rows per core
NKB = S // P         # 16 k-blocks
NQT = QH // P        # 8 q-tiles per core
NQC = QH // 512      # 2 q-chunks of 512
XTW = H * (DK + 1)   # 520: token-major row width incl. ones columns
HS = DK + 1          # per-head stride in xtok

F32 = mybir.dt.float32
F16 = mybir.dt.float16
BF16 = mybir.dt.bfloat16
I32 = mybir.dt.int32
AF = mybir.ActivationFunctionType
ALU = mybir.AluOpType


def build(n_layers=N_LAYERS, use_affine=False):
    nc = bacc.Bacc("TRN2", target_bir_lowering=False, debug=False, num_devices=NC)

    # ---- I/O ----
    x_feat = nc.declare_dram_parameter("x_feat", [D, S], F16, isOutput=False)
    x_tok = nc.declare_dram_parameter("x_tok", [S, XTW], BF16, isOutput=False)
    x_own = nc.declare_dram_parameter("x_own", [QH, D], F16, isOutput=False)
    idxf_in = nc.declare_dram_parameter("idxf", [P, D // P], I32, isOutput=False)
    idxt_in = nc.declare_dram_parameter("idxt", [P, NQT], I32, isOutput=False)
    # woT blocked [l, oc, 128, D]: rhs tiles for the wo matmul
    woT_d = nc.declare_dram_parameter("woT", [n_layers, D // P, P, D], F16, isOutput=False)
    # w1T blocked [l, jc, 128(p=d), 4(oc), 128(j)]: one [128,512] tile per jc
    w1T_d = nc.declare_dram_parameter(
        "w1T", [n_layers, DFF // P, P, D // P, P], F16, isOutput=False)
    # w2T blocked [l, jc, 128(p=j), D(o)]
    w2T_d = nc.declare_dram_parameter("w2T", [n_layers, DFF // P, P, D], F16, isOutput=False)
    wob_d = nc.declare_dram_parameter("wob", [n_layers, D], F32, isOutput=False)
    fc1b_d = nc.declare_dram_parameter("fc1b", [n_layers, DFF], F32, isOutput=False)
    fc2b_d = nc.declare_dram_parameter("fc2b", [n_layers, D], F32, isOutput=False)
    g1_d = nc.declare_dram_parameter("g1", [n_layers, D], F32, isOutput=False)
    b1_d = nc.declare_dram_parameter("b1", [n_layers, D], F32, isOutput=False)
    g2_d = nc.declare_dram_parameter("g2", [n_layers, D], F32, isOutput=False)
    b2_d = nc.declare_dram_parameter("b2", [n_layers, D], F32, isOutput=False)
    out_d = nc.declare_dram_parameter("out", [QH, D], F16, isOutput=True)

    # collective staging: xT own half [4, 128, 1024] f16 + xtok own [8, 128, 520] bf16
    agf_ins = [nc.dram_tensor(f"agf_in{l}", [D // P, P, QH], F16) for l in range(n_layers - 1)]
    agt_ins = [nc.dram_tensor(f"agt_in{l}", [NQT, P, XTW], BF16) for l in range(n_layers - 1)]
    agf_outs = [
        nc.dram_tensor(f"agf_out{l}", [2, D // P, P, QH], F16)
        for l in range(n_layers - 1)
    ]
    agt_outs = [
        nc.dram_tensor(f"agt_out{l}", [2, NQT, P, XTW], BF16)
        for l in range(n_layers - 1)
    ]

    with nc.allow_low_precision(reason="deliberate f16/bf16 compute"), tile.TileContext(nc) as tc:
        from contextlib import ExitStack

        with ExitStack() as ctx:
            persist = ctx.enter_context(tc.tile_pool(name="persist", bufs=1))
            wo_pool = ctx.enter_context(tc.tile_pool(name="wo_pool", bufs=5))
            w1_pool = ctx.enter_context(tc.tile_pool(name="w1_pool", bufs=4))
            w2_pool = ctx.enter_context(tc.tile_pool(name="w2_pool", bufs=16))
            b_pool = ctx.enter_context(tc.tile_pool(name="b_pool", bufs=7))
            fb_pool = ctx.enter_context(tc.tile_pool(name="fb_pool", bufs=2))
            e_pool = ctx.enter_context(tc.tile_pool(name="e_pool", bufs=3))
            h_pool = ctx.enter_context(tc.tile_pool(name="h_pool", bufs=16))
            work = ctx.enter_context(tc.tile_pool(name="work", bufs=4))
            small = ctx.enter_context(tc.tile_pool(name="small", bufs=6))
            op_pool = ctx.enter_context(tc.tile_pool(name="op_pool", bufs=8))
            # PSUM budget (16KB/partition): duo 2x4KB + facc 1x4KB + pv 2x2KB
            ps = ctx.enter_context(tc.tile_pool(name="ps", bufs=2, space="PSUM"))

            # ---- persistent state ----
            xT = [persist.tile([P, S], F16, tag=f"xT{i}", name=f"xT{i}") for i in range(D // P)]
            xtok = [persist.tile([P, XTW], BF16, tag=f"xtok{i}", name=f"xtok{i}") for i in range(NKB)]
            xres = [persist.tile([P, D], F16, tag=f"xres{i}", name=f"xres{i}") for i in range(NQT)]
            x1 = [persist.tile([P, D], F16, tag=f"x1_{i}", name=f"x1_{i}") for i in range(NQT)]
            x1T = [persist.tile([P, QH], F16, tag=f"x1T{i}", name=f"x1T{i}") for i in range(D // P)]
            oT = [persist.tile([P, QH], F16, tag=f"oT{i}", name=f"oT{i}") for i in range(D // P)]
            identh = persist.tile([P, P], F16, tag="identh")
            identf = persist.tile([P, P], F32, tag="identf")
            ones1h = persist.tile([1, DK], F16, tag="ones1h")
            epsT = persist.tile([P, 1], F32, tag="epsT")
            # partial softmax denominators: one [1, 1024] tile per (qc, pair)
            # group (hp0 in cols 0:512, hp1 in 512:1024; partition 0 only —
            # partition bases must be 32-aligned so no [2, ...] row tiles)
            sparts = [persist.tile([1, 1024], F32, tag=f"sp{g}", name=f"sp{g}")
                      for g in range(NQC * 4)]
            idxf_sb = persist.tile([P, D // P], I32, tag="idxf_sb")
            idxt_sb = persist.tile([P, NQT], I32, tag="idxt_sb")

            make_identity(nc, identf[:])
            nc.vector.tensor_copy(out=identh[:], in_=identf[:])
            nc.vector.memset(ones1h[:], 1.0)
            nc.vector.memset(epsT[:], EPS)
            nc.sync.dma_start(idxf_sb[:], idxf_in[:])
            nc.sync.dma_start(idxt_sb[:], idxt_in[:])

            # ---- initial loads ----
            for i in range(D // P):
                nc.sync.dma_start(xT[i][:], x_feat[ts(i, P), :])
            for i in range(NKB):
                nc.sync.dma_start(xtok[i][:], x_tok[ts(i, P), :])
            for i in range(NQT):
                nc.sync.dma_start(xres[i][:], x_own[ts(i, P), :])

            def load_layer_weights(l):
                woT_sb = []
                for oc in range(D // P):
                    t = wo_pool.tile([P, D], F16, tag="woT_sb", name="woT_sb")
                    nc.sync.dma_start(t[:], woT_d[l, oc])
                    woT_sb.append(t)
                bc = {}
                if use_affine:
                    for name, dram in (
                        ("wob", wob_d), ("fc2b", fc2b_d),
                        ("g1", g1_d), ("b1", b1_d), ("g2", g2_d), ("b2", b2_d),
                    ):
                        t = b_pool.tile([P, D], F32, tag="bc", name="bc")
                        nc.sync.dma_start(t[:], dram[l, None, :].to_broadcast((P, D)))
                        bc[name] = t
                    fc1b_sb = fb_pool.tile([P, DFF // P], F32, tag="fc1b_sb")
                    nc.sync.dma_start(
                        fc1b_sb[:], fc1b_d[l, :].rearrange("(a p) -> p a", p=P)
                    )
                else:
                    fc1b_sb = None
                return woT_sb, bc, fc1b_sb

            def recip_fast(out_ap, in_ap):
                from concourse.dve_ops import (
                    RECIP_APPROX_FAST_CONSTS,
                    RECIPROCAL_APPROX_FAST,
                )

                c = RECIP_APPROX_FAST_CONSTS
                nc.vector._custom_dve(
                    RECIPROCAL_APPROX_FAST, out=out_ap, in0=in_ap,
                    s0=c["s0"], s1=c["s1"], imm2=c["imm2"],
                )

            def attn_kb(pair, qc, pv, kb, start, stop):
                """One k-block: 2 scores MMs + exp + 2 PV MMs."""
                q0 = qc * 512
                duo = ps.tile([P, 1024], F32, tag="duo", name="duo")
                for hp in range(2):
                    nc.tensor.matmul(
                        duo[:, ts(hp, 512)],
                        xT[pair][ts(hp, DK), ts(kb, P)],
                        xT[pair][ts(hp, DK), ds(q0, 512)],
                        start=True, stop=True,
                    )
                e_t = e_pool.tile([P, 1024], BF16, tag="e", name="e_t")
                nc.scalar.activation(e_t[:], duo[:], AF.Exp, scale=1.0 / np.sqrt(DK))
                for hp in range(2):
                    h = 2 * pair + hp
                    nc.tensor.matmul(
                        pv[hp][0 : DK + 1, :],
                        xtok[kb][:, ds(h * HS, DK + 1)], e_t[:, ts(hp, 512)],
                        start=start, stop=stop,
                    )

            def attention_part1(l, pair, qc):
                """Own-half k-blocks; evicts partial O (bf16 SBUF) and the
                partial rowsum rows (SBUF scratch), freeing PSUM while the
                AllGather completes in the background."""
                pv = [ps.tile([P, 512], F32, tag="pv", name="pv") for _ in range(2)]
                for kb in range(NKB // 2):
                    attn_kb(pair, qc, pv, kb, kb == 0, kb == NKB // 2 - 1)
                g = qc * 4 + pair
                opart = op_pool.tile([P, 512], BF16, tag="opart", name="opart")
                for hp in range(2):
                    nc.vector.tensor_copy(
                        out=opart[ts(hp, DK), :], in_=pv[hp][0:DK, :]
                    )
                    nc.vector.tensor_copy(
                        out=sparts[g][0:1, ts(hp, 512)], in_=pv[hp][DK : DK + 1, :]
                    )
                return opart

            def part2_epilogue(pair, qc, pv, opart):
                """Combine peer-half PV with part1 + normalize into oT."""
                q0 = qc * 512
                g = qc * 4 + pair
                s2 = small.tile([1, 1024], F32, tag="s_sb", name="s2")
                for hp in range(2):
                    nc.vector.tensor_add(
                        s2[0:1, ts(hp, 512)], pv[hp][DK : DK + 1, :],
                        sparts[g][0:1, ts(hp, 512)],
                    )
                rf = small.tile([1, 1024], F32, tag="s_sb", name="rf")
                recip_fast(rf[:], s2[:])
                r2 = small.tile([1, 1024], F16, tag="r2", name="r2")
                nc.vector.tensor_copy(out=r2[:], in_=rf[:])
                bc_ps = ps.tile([P, 512], F32, tag="pv", name="bc_ps")
                for hp in range(2):
                    nc.tensor.matmul(
                        bc_ps[ts(hp, DK), :], ones1h[:], r2[0:1, ts(hp, 512)],
                        start=True, stop=True,
                    )
                for hp in range(2):
                    o_un = work.tile([P, 512], F32, tag="work", name="o_un")
                    nc.vector.tensor_add(
                        o_un[0:DK, :], pv[hp][0:DK, :], opart[ts(hp, DK), :]
                    )
                    dst = oT[pair][ts(hp, DK), ds(q0, 512)]
                    nc.vector.tensor_mul(dst, o_un[0:DK, :], bc_ps[ts(hp, DK), :])

            def ln_stats(src_tile):
                stats = small.tile([P, 6], F32, tag="stats")
                nc.vector.bn_stats(out=stats[:], in_=src_tile[:])
                mv = small.tile([P, 2], F32, tag="mv")
                nc.vector.bn_aggr(out=mv[:], in_=stats[:])
                return mv

            def ln_rstd_batch(mvs):
                n = len(mvs)
                vb = small.tile([P, 8], F32, tag="vb", name="vb")
                for i, mv in enumerate(mvs):
                    nc.vector.tensor_copy(out=vb[:, i : i + 1], in_=mv[:, 1:2])
                nc.scalar.activation(
                    out=vb[:, :n], in_=vb[:, :n], func=AF.Sqrt,
                    bias=epsT[:], scale=1.0,
                )
                nc.vector.reciprocal_approx_fast(out=vb[:, :n], in_=vb[:, :n])
                return vb

            def ln_apply(dst, src_tile, mv, rstd1, g_bc, b_bc):
                nc.vector.tensor_scalar(
                    out=dst[:], in0=src_tile[:],
                    scalar1=mv[:, 0:1], scalar2=rstd1,
                    op0=ALU.subtract, op1=ALU.mult,
                )
                if use_affine:
                    nc.vector.tensor_mul(dst[:], dst[:], g_bc[:])
                    nc.vector.tensor_add(dst[:], dst[:], b_bc[:])

            def wo_ln1(l, half, woT_sb, bc):
                ts_, mvs = [], []
                for q4 in range(4):
                    qt = half * 4 + q4
                    y_ps = ps.tile([P, D], F32, tag="duo", name="y_ps")
                    for oc in range(D // P):
                        nc.tensor.matmul(
                            y_ps[:], oT[oc][:, ts(qt, P)], woT_sb[oc][:],
                            start=(oc == 0), stop=(oc == D // P - 1),
                        )
                    t = work.tile([P, D], F32, tag="work")
                    nc.vector.tensor_add(t[:], y_ps[:], xres[qt][:])
                    if use_affine:
                        nc.vector.tensor_add(t[:], t[:], bc["wob"][:])
                    ts_.append(t)
                    mvs.append(ln_stats(t))
                vb = ln_rstd_batch(mvs)
                for q4 in range(4):
                    qt = half * 4 + q4
                    ln_apply(x1[qt], ts_[q4], mvs[q4], vb[:, q4 : q4 + 1],
                             bc.get("g1"), bc.get("b1"))
                    for ft in range(D // P):
                        tp = ps.tile([P, P], F16, tag="duo", name="tp")
                        nc.tensor.transpose(tp[:], x1[qt][:, ts(ft, P)], identh[:])
                        nc.vector.tensor_copy(out=x1T[ft][:, ts(qt, P)], in_=tp[:])

            def ffn_s1_step(l, qc, jc, facc01, hTs, w2s, fc1b_sb):
                """fc1(jc) + hT store + fc2 into q-tiles 0,1."""
                h_ps = ps.tile([P, 512], F32, tag="duo", name="h_ps")
                w1c = w1_pool.tile([P, D], F16, tag="w1c", name="w1c")
                nc.sync.dma_start(
                    w1c[:], w1T_d[l, jc].rearrange("p a j -> p (a j)")
                )
                for oc in range(D // P):
                    nc.tensor.matmul(
                        h_ps[:], w1c[:, ts(oc, P)],
                        x1T[oc][:, ds(qc * 512, 512)],
                        start=(oc == 0), stop=(oc == D // P - 1),
                    )
                hT = h_pool.tile([P, 512], F16, tag="hT", name="hT")
                if use_affine:
                    nc.vector.tensor_scalar_add(hT[:], h_ps[:], fc1b_sb[:, jc : jc + 1])
                else:
                    nc.vector.tensor_copy(out=hT[:], in_=h_ps[:])
                hTs.append(hT)
                w2c = w2_pool.tile([P, D], F16, tag="w2c", name="w2c")
                nc.sync.dma_start(w2c[:], w2T_d[l, jc])
                w2s.append(w2c)
                for q4 in range(2):
                    nc.tensor.matmul(
                        facc01[:, ts(q4, 512)], hTs[jc][:, ts(q4, P)], w2s[jc][:],
                        start=(jc == 0), stop=(jc == DFF // P - 1),
                    )

            def ffn_s2_step(l, qc, jc, facc23, hTs, w2s):
                for q4 in range(2, 4):
                    nc.tensor.matmul(
                        facc23[:, ts(q4 - 2, 512)], hTs[jc][:, ts(q4, P)], w2s[jc][:],
                        start=(jc == 0), stop=(jc == DFF // P - 1),
                    )

            def ffn_t2(qc, q4, facc_half, bc):
                qt = qc * 4 + q4
                t2 = work.tile([P, D], F32, tag="work")
                nc.vector.tensor_add(t2[:], facc_half[:, ts(q4 % 2, 512)], x1[qt][:])
                if use_affine:
                    nc.vector.tensor_add(t2[:], t2[:], bc["fc2b"][:])
                return t2, ln_stats(t2)

            def ffn_finish(l, qc, t2s, mvs2, bc):
                vb2 = ln_rstd_batch(mvs2)
                for q4 in range(4):
                    qt = qc * 4 + q4
                    ln_apply(xres[qt], t2s[q4], mvs2[q4],
                             vb2[:, q4 : q4 + 1], bc.get("g2"), bc.get("b2"))
                    if l == n_layers - 1:
                        nc.sync.dma_start(out_d[ts(qt, P), :], xres[qt][:])

            def rebuild_qt(l, i):
                """xtok tile i and xT col-block i from xres[i] (= new x)."""
                src3 = xres[i][:].rearrange("p (h k) -> p h k", k=DK)
                dst3 = xtok[i][:].rearrange("p (h k) -> p h k", k=HS)
                nc.vector.tensor_copy(out=dst3[:, :, 0:DK], in_=src3)
                nc.sync.dma_start(agt_ins[l][i], xtok[i][:])
                for ft in range(D // P):
                    tp = ps.tile([P, P], F16, tag="duo", name="tp")
                    nc.tensor.transpose(tp[:], xres[i][:, ts(ft, P)], identh[:])
                    nc.vector.tensor_copy(out=xT[ft][:, ts(i, P)], in_=tp[:])

            def fetch_peer(l):
                """xT cols 1024:2048 / xtok tiles 8..15 straight from the
                AllGather output — no rebuild compute on the receiver."""
                agf_flat = agf_outs[l][:].rearrange("r f p q -> (r f p) q")
                for ft in range(D // P):
                    nc.gpsimd.indirect_dma_start(
                        out=xT[ft][:, QH:S],
                        out_offset=None,
                        in_=agf_flat,
                        in_offset=bass.IndirectOffsetOnAxis(
                            ap=idxf_sb[:, ft : ft + 1], axis=0
                        ),
                    )
                agt_flat = agt_outs[l][:].rearrange("r i p w -> (r i p) w")
                for i in range(NQT):
                    nc.gpsimd.indirect_dma_start(
                        out=xtok[NQT + i][:],
                        out_offset=None,
                        in_=agt_flat,
                        in_offset=bass.IndirectOffsetOnAxis(
                            ap=idxt_sb[:, i : i + 1], axis=0
                        ),
                    )

            def emit_part1(l):
                oparts = {}
                for qc in range(NQC):
                    for pair in range(4):
                        oparts[(qc, pair)] = attention_part1(l, pair, qc)
                return oparts

            def part2_alone(l, qc, oparts):
                for pair in range(4):
                    pv = [ps.tile([P, 512], F32, tag="pv", name="pv") for _ in range(2)]
                    for kb_i in range(NKB // 2):
                        attn_kb(pair, qc, pv, NKB // 2 + kb_i,
                                kb_i == 0, kb_i == NKB // 2 - 1)
                    part2_epilogue(pair, qc, pv, oparts[(qc, pair)])

            def ffn_sweeps(l, qc, bc, fc1b_sb, attn=None, rebuild=False):
                """Two-sweep FFN for q-chunk qc. If `attn` is (qc_a, oparts),
                part2(qc_a) k-block steps interleave 2:1 with sweep-1 jc
                steps so softmax exps keep the scalar engine fed while the
                PE runs FFN matmuls. If `rebuild`, rebuild_qt(0..3) steps
                interleave instead (their xres came from the prior qc)."""
                facc01 = ps.tile([P, 1024], F32, tag="facc", name="facc01", bufs=1)
                hTs, w2s = [], []
                jc = 0
                if attn is not None:
                    qc_a, oparts = attn
                    for pair in range(4):
                        pv = [ps.tile([P, 512], F32, tag="pv", name="pv") for _ in range(2)]
                        for kb_i in range(NKB // 2):
                            attn_kb(pair, qc_a, pv, NKB // 2 + kb_i,
                                    kb_i == 0, kb_i == NKB // 2 - 1)
                            if kb_i % 2 == 1:
                                ffn_s1_step(l, qc, jc, facc01, hTs, w2s, fc1b_sb)
                                jc += 1
                        part2_epilogue(pair, qc_a, pv, oparts[(qc_a, pair)])
                while jc < DFF // P:
                    ffn_s1_step(l, qc, jc, facc01, hTs, w2s, fc1b_sb)
                    if rebuild and jc % 4 == 3 and l < n_layers - 1:
                        rebuild_qt(l, jc // 4)
                    jc += 1
                t2s, mvs2 = [], []
                for q4 in range(2):
                    t2, mv = ffn_t2(qc, q4, facc01, bc)
                    t2s.append(t2); mvs2.append(mv)
                facc23 = ps.tile([P, 1024], F32, tag="facc", name="facc23", bufs=1)
                for jc in range(DFF // P):
                    ffn_s2_step(l, qc, jc, facc23, hTs, w2s)
                for q4 in range(2, 4):
                    t2, mv = ffn_t2(qc, q4, facc23, bc)
                    t2s.append(t2); mvs2.append(mv)
                ffn_finish(l, qc, t2s, mvs2, bc)

            def ffn_plain(l, qc, bc, fc1b_sb):
                facc = [ps.tile([P, 1024], F32, tag="duo", name="facc") for _ in range(2)]
                for jc in range(DFF // P):
                    h_ps = ps.tile([P, 512], F32, tag="pv", name="h_ps")
                    w1c = w1_pool.tile([P, D], F16, tag="w1c", name="w1c")
                    nc.sync.dma_start(
                        w1c[:], w1T_d[l, jc].rearrange("p a j -> p (a j)")
                    )
                    for oc in range(D // P):
                        nc.tensor.matmul(
                            h_ps[:], w1c[:, ts(oc, P)],
                            x1T[oc][:, ds(qc * 512, 512)],
                            start=(oc == 0), stop=(oc == D // P - 1),
                        )
                    hT = h_pool.tile([P, 512], F16, tag="hT", name="hT")
                    if use_affine:
                        nc.vector.tensor_scalar_add(hT[:], h_ps[:], fc1b_sb[:, jc : jc + 1])
                    else:
                        nc.vector.tensor_copy(out=hT[:], in_=h_ps[:])
                    w2c = w2_pool.tile([P, D], F16, tag="w2c", name="w2c")
                    nc.sync.dma_start(w2c[:], w2T_d[l, jc])
                    for q4 in range(4):
                        nc.tensor.matmul(
                            facc[q4 // 2][:, ts(q4 % 2, 512)], hT[:, ts(q4, P)], w2c[:],
                            start=(jc == 0), stop=(jc == DFF // P - 1),
                        )
                t2s, mvs2 = [], []
                for q4 in range(4):
                    t2, mv = ffn_t2(qc, q4, facc[q4 // 2], bc)
                    t2s.append(t2); mvs2.append(mv)
                ffn_finish(l, qc, t2s, mvs2, bc)

            # ---- the stack ----
            oparts = None
            for l in range(n_layers):
                woT_sb, bc, fc1b_sb = load_layer_weights(l)
                if l == 0:
                    oparts = emit_part1(0)
                part2_alone(l, 0, oparts)
                part2_alone(l, 1, oparts)
                wo_ln1(l, 0, woT_sb, bc)
                wo_ln1(l, 1, woT_sb, bc)
                ffn_plain(l, 0, bc, fc1b_sb)
                ffn_plain(l, 1, bc, fc1b_sb)
                if l < n_layers - 1:
                    for i in range(NQT):
                        rebuild_qt(l, i)
                    for ft in range(D // P):
                        nc.sync.dma_start(agf_ins[l][ft], xT[ft][:, 0:QH])
                    nc.gpsimd.collective_compute(
                        "AllGather", ALU.bypass,
                        ins=[agf_ins[l][:].opt()],
                        outs=[agf_outs[l][:].opt()],
                        replica_groups=[[0, 1], [2, 3], [4, 5], [6, 7]],
                    )
                    nc.gpsimd.collective_compute(
                        "AllGather", ALU.bypass,
                        ins=[agt_ins[l][:].opt()],
                        outs=[agt_outs[l][:].opt()],
                        replica_groups=[[0, 1], [2, 3], [4, 5], [6, 7]],
                    )
                    oparts = emit_part1(l + 1)
                    fetch_peer(l)

    nc.compile()
    return nc


# ---- host side ----

_cache = {}


def _get_nc(n_layers=N_LAYERS, use_affine=False):
    key = (n_layers, use_affine)
    if key not in _cache:
        _cache[key] = build(n_layers, use_affine)
    return _cache[key]


def _trivial_affine(inputs, n_layers):
    return (
        not np.any(np.asarray(inputs["wo_b"], np.float32)[:n_layers])
        and not np.any(np.asarray(inputs["fc1_b"], np.float32)[:n_layers])
        and not np.any(np.asarray(inputs["fc2_b"], np.float32)[:n_layers])
        and not np.any(np.asarray(inputs["ln1_b"], np.float32)[:n_layers])
        and not np.any(np.asarray(inputs["ln2_b"], np.float32)[:n_layers])
        and np.all(np.asarray(inputs["ln1_g"], np.float32)[:n_layers] == 1.0)
        and np.all(np.asarray(inputs["ln2_g"], np.float32)[:n_layers] == 1.0)
    )


def make_in_maps(inputs, n_layers=N_LAYERS):
    f16 = ml_dtypes.float16 if hasattr(ml_dtypes, "float16") else np.float16
    x = np.asarray(inputs["x"], dtype=np.float32)
    woT = np.asarray(inputs["wo_w"], np.float32)[:n_layers].transpose(0, 2, 1)
    woT = np.ascontiguousarray(
        woT.reshape(n_layers, D // P, P, D)).astype(np.float16)
    w1T = np.asarray(inputs["fc1_w"], np.float32)[:n_layers].transpose(0, 2, 1)
    # [l, d, j] -> [l, jc, p(d-part within oc? no: p is d%?)]
    # desired tile[l, jc, p, oc, jj] = w1T[l, oc*128+p, jc*128+jj]
    w1T = w1T.reshape(n_layers, D // P, P, DFF // P, P).transpose(0, 3, 2, 1, 4)
    w1T = np.ascontiguousarray(w1T).astype(np.float16)
    w2T = np.asarray(inputs["fc2_w"], np.float32)[:n_layers].transpose(0, 2, 1)
    w2T = np.ascontiguousarray(
        w2T.reshape(n_layers, DFF // P, P, D)).astype(np.float16)
    common = {
        "woT": woT, "w1T": w1T, "w2T": w2T,
        "wob": np.ascontiguousarray(np.asarray(inputs["wo_b"], np.float32)[:n_layers]),
        "fc1b": np.ascontiguousarray(np.asarray(inputs["fc1_b"], np.float32)[:n_layers]),
        "fc2b": np.ascontiguousarray(np.asarray(inputs["fc2_b"], np.float32)[:n_layers]),
        "g1": np.ascontiguousarray(np.asarray(inputs["ln1_g"], np.float32)[:n_layers]),
        "b1": np.ascontiguousarray(np.asarray(inputs["ln1_b"], np.float32)[:n_layers]),
        "g2": np.ascontiguousarray(np.asarray(inputs["ln2_g"], np.float32)[:n_layers]),
        "b2": np.ascontiguousarray(np.asarray(inputs["ln2_b"], np.float32)[:n_layers]),
    }
    in_maps = []
    for c in range(NC):
        b, half = c // 2, c % 2
        own = x[b, half * QH : (half + 1) * QH]        # [QH, D]
        peer = x[b, (1 - half) * QH : (2 - half) * QH]
        local = np.concatenate([own, peer], axis=0)     # [S, D] core-relative
        x_feat = np.ascontiguousarray(local.T).astype(np.float16)  # [D, S]
        xt = np.zeros((S, H, HS), np.float32)
        xt[:, :, :DK] = local.reshape(S, H, DK)
        xt[:, :, DK] = 1.0
        x_tok = xt.reshape(S, XTW).astype(ml_dtypes.bfloat16)
        # peer slot within the 2-rank AllGather group
        pr = 1 - half
        p_ar = np.arange(P, dtype=np.int32)
        idxf = np.ascontiguousarray(
            (pr * (D // P) * P + np.arange(D // P, dtype=np.int32)[None, :] * P
             + p_ar[:, None]))
        idxt = np.ascontiguousarray(
            (pr * NQT * P + np.arange(NQT, dtype=np.int32)[None, :] * P
             + p_ar[:, None]))
        m = dict(common)
        m.update({
            "x_feat": x_feat, "x_tok": x_tok,
            "x_own": np.ascontiguousarray(own).astype(np.float16),
            "idxf": idxf, "idxt": idxt,
        })
        in_maps.append(m)
    return in_maps


def assemble_output(results):
    out = np.empty((B, S, D), np.float32)
    for c in range(NC):
        b, half = c // 2, c % 2
        out[b, half * QH : (half + 1) * QH] = np.asarray(
            results[c]["out"], dtype=np.float32)
    return out


def kernel(**inputs):
    from concourse.bass_utils import run_bass_kernel_spmd

    use_affine = not _trivial_affine(inputs, N_LAYERS)
    nc = _get_nc(N_LAYERS, use_affine)
    in_maps = make_in_maps(inputs)
    res = run_bass_kernel_spmd(nc, in_maps, core_ids=list(range(NC)))
    return assemble_output(res.results)


# revision 18
# speedup vs baseline: 1.1499x; 1.1282x over previous
"""Trainium2 Bass kernel for nn_Encoder_73778948211333.

6-layer transformer encoder (no qkv projections: q=k=v=head slices of x),
B=4, S=2048, D=512, H=8 heads, DFF=2048, fp32, no activation between fc1/fc2.

Sharding: 8 cores = (batch, sequence-half). Each core owns 1024 query rows of
one batch: computes attention for its rows (k-major scores -> exp -> PV with a
fused ones-column rowsum), wo + LN1 + FFN + LN2 for its rows, then a PAIRWISE
AllGather ([0,1],[2,3],...) exchanges updated halves between layers. The
payload carries the already-transposed feature-major tiles (xT, fp16) and the
token-major value tiles (xtok, bf16) so the receiver does zero rebuild
compute — peer tiles land via indirect row-gather DMAs.

All matmuls run in fp16/bf16 (full PE rate with fast-weight-load; the fp32
"HIGH" mode the previous version used streams at less than half rate on real
silicon). Softmax skips max-subtraction: scores are bounded since every layer
output is layer-normalized; exp outputs bf16 (large dynamic range).
LN gains/biases that are exactly 1/0 in the inputs are skipped at build time
(checked host-side; a general build is used otherwise).
"""

import sys

sys.path.insert(0, "/opt/trn_rl_repo")
sys.path.insert(0, "/root/.axon_site")

import numpy as np
import ml_dtypes

import concourse.bass as bass
import concourse.tile as tile
from concourse import bacc, mybir
from concourse.bass import ds, ts
from concourse.masks import make_identity

# ---- problem constants (hardcoded per spec) ----
B, S, D = 4, 2048, 512
H, DK = 8, 64
DFF = 4 * D
N_LAYERS = 6
EPS = 1e-8
P = 128
NC = 8
QH = S // 2          # 1024 rows per core
NKB = S // P         # 16 k-blocks
NQT = QH // P        # 8 q-tiles per core
NQC = QH // 512      # 2 q-chunks of 512
XTW = H * (DK + 1)   # 520: token-major row width incl. ones columns
HS = DK + 1          # per-head stride in xtok

F32 = mybir.dt.float32
F16 = mybir.dt.float16
BF16 = mybir.dt.bfloat16
I32 = mybir.dt.int32
AF = mybir.ActivationFunctionType
ALU = mybir.AluOpType


def build(n_layers=N_LAYERS, use_affine=False):
    nc = bacc.Bacc("TRN2", target_bir_lowering=False, debug=False, num_devices=NC)

    # ---- I/O ----
    x_feat = nc.declare_dram_parameter("x_feat", [D, S], F16, isOutput=False)
    x_tok = nc.declare_dram_parameter("x_tok", [S, XTW], BF16, isOutput=False)
    x_own = nc.declare_dram_parameter("x_own", [QH, D], F16, isOutput=False)
    idxf_in = nc.declare_dram_parameter("idxf", [P, D // P], I32, isOutput=False)
    idxt_in = nc.declare_dram_parameter("idxt", [P, NQT], I32, isOutput=False)
    # woT blocked [l, oc, 128, D]: rhs tiles for the wo matmul
    woT_d = nc.declare_dram_parameter("woT", [n_layers, D // P, P, D], F16, isOutput=False)
    # w1T blocked [l, jc, 128(p=d), 4(oc), 128(j)]: one [128,512] tile per jc
    w1T_d = nc.declare_dram_parameter(
        "w1T", [n_layers, DFF // P, P, D // P, P], F16, isOutput=False)
    # w2T blocked [l, jc, 128(p=j), D(o)]
    w2T_d = nc.declare_dram_parameter("w2T", [n_layers, DFF // P, P, D], F16, isOutput=False)
    wob_d = nc.declare_dram_parameter("wob", [n_layers, D], F32, isOutput=False)
    fc1b_d = nc.declare_dram_parameter("fc1b", [n_layers, DFF], F32, isOutput=False)
    fc2b_d = nc.declare_dram_parameter("fc2b", [n_layers, D], F32, isOutput=False)
    g1_d = nc.declare_dram_parameter("g1", [n_layers, D], F32, isOutput=False)
    b1_d = nc.declare_dram_parameter("b1", [n_layers, D], F32, isOutput=False)
    g2_d = nc.declare_dram_parameter("g2", [n_layers, D], F32, isOutput=False)
    b2_d = nc.declare_dram_parameter("b2", [n_layers, D], F32, isOutput=False)
    out_d = nc.declare_dram_parameter("out", [QH, D], F16, isOutput=True)

    # collective staging: xT own half [4, 128, 1024] f16 + xtok own [8, 128, 520] bf16
    agf_ins = [nc.dram_tensor(f"agf_in{l}", [D // P, P, QH], F16) for l in range(n_layers - 1)]
    agt_ins = [nc.dram_tensor(f"agt_in{l}", [NQT, P, XTW], BF16) for l in range(n_layers - 1)]
    agf_outs = [
        nc.dram_tensor(f"agf_out{l}", [2, D // P, P, QH], F16)
        for l in range(n_layers - 1)
    ]
    agt_outs = [
        nc.dram_tensor(f"agt_out{l}", [2, NQT, P, XTW], BF16)
        for l in range(n_layers - 1)
    ]

    with nc.allow_low_precision(reason="deliberate f16/bf16 compute"), tile.TileContext(nc) as tc:
        from contextlib import ExitStack

        with ExitStack() as ctx:
            persist = ctx.enter_context(tc.tile_pool(name="persist", bufs=1))
            wo_pool = ctx.enter_context(tc.tile_pool(name="wo_pool", bufs=5))
            w1_pool = ctx.enter_context(tc.tile_pool(name="w1_pool", bufs=4))
            w2_pool = ctx.enter_context(tc.tile_pool(name="w2_pool", bufs=16))
            b_pool = ctx.enter_context(tc.tile_pool(name="b_pool", bufs=7))
            fb_pool = ctx.enter_context(tc.tile_pool(name="fb_pool", bufs=2))
            e_pool = ctx.enter_context(tc.tile_pool(name="e_pool", bufs=3))
            h_pool = ctx.enter_context(tc.tile_pool(name="h_pool", bufs=16))
            work = ctx.enter_context(tc.tile_pool(name="work", bufs=4))
            small = ctx.enter_context(tc.tile_pool(name="small", bufs=6))
            op_pool = ctx.enter_context(tc.tile_pool(name="op_pool", bufs=8))
            # PSUM budget (16KB/partition): duo 2x4KB + facc 1x4KB + pv 2x2KB
            ps = ctx.enter_context(tc.tile_pool(name="ps", bufs=2, space="PSUM"))

            # ---- persistent state ----
            xT = [persist.tile([P, S], F16, tag=f"xT{i}", name=f"xT{i}") for i in range(D // P)]
            xtok = [persist.tile([P, XTW], BF16, tag=f"xtok{i}", name=f"xtok{i}") for i in range(NKB)]
            xres = [persist.tile([P, D], F16, tag=f"xres{i}", name=f"xres{i}") for i in range(NQT)]
            x1 = [persist.tile([P, D], F16, tag=f"x1_{i}", name=f"x1_{i}") for i in range(NQT)]
            x1T = [persist.tile([P, QH], F16, tag=f"x1T{i}", name=f"x1T{i}") for i in range(D // P)]
            oT = [persist.tile([P, QH], F16, tag=f"oT{i}", name=f"oT{i}") for i in range(D // P)]
            identh = persist.tile([P, P], F16, tag="identh")
            identf = persist.tile([P, P], F32, tag="identf")
            ones1h = persist.tile([1, DK], F16, tag="ones1h")
            epsT = persist.tile([P, 1], F32, tag="epsT")
            # partial softmax denominators: one [1, 1024] tile per (qc, pair)
            # group (hp0 in cols 0:512, hp1 in 512:1024; partition 0 only —
            # partition bases must be 32-aligned so no [2, ...] row tiles)
            sparts = [persist.tile([1, 1024], F32, tag=f"sp{g}", name=f"sp{g}")
                      for g in range(NQC * 4)]
            idxf_sb = persist.tile([P, D // P], I32, tag="idxf_sb")
            idxt_sb = persist.tile([P, NQT], I32, tag="idxt_sb")

            make_identity(nc, identf[:])
            nc.vector.tensor_copy(out=identh[:], in_=identf[:])
            nc.vector.memset(ones1h[:], 1.0)
            nc.vector.memset(epsT[:], EPS)
            nc.sync.dma_start(idxf_sb[:], idxf_in[:])
            nc.sync.dma_start(idxt_sb[:], idxt_in[:])

            # ---- initial loads ----
            for i in range(D // P):
                nc.sync.dma_start(xT[i][:], x_feat[ts(i, P), :])
            for i in range(NKB):
                nc.sync.dma_start(xtok[i][:], x_tok[ts(i, P), :])
            for i in range(NQT):
                nc.sync.dma_start(xres[i][:], x_own[ts(i, P), :])

            def load_layer_weights(l):
                woT_sb = []
                for oc in range(D // P):
                    t = wo_pool.tile([P, D], F16, tag="woT_sb", name="woT_sb")
                    nc.sync.dma_start(t[:], woT_d[l, oc])
                    woT_sb.append(t)
                bc = {}
                if use_affine:
                    for name, dram in (
                        ("wob", wob_d), ("fc2b", fc2b_d),
                        ("g1", g1_d), ("b1", b1_d), ("g2", g2_d), ("b2", b2_d),
                    ):
                        t = b_pool.tile([P, D], F32, tag="bc", name="bc")
                        nc.sync.dma_start(t[:], dram[l, None, :].to_broadcast((P, D)))
                        bc[name] = t
                    fc1b_sb = fb_pool.tile([P, DFF // P], F32, tag="fc1b_sb")
                    nc.sync.dma_start(
                        fc1b_sb[:], fc1b_d[l, :].rearrange("(a p) -> p a", p=P)
                    )
                else:
                    fc1b_sb = None
                return woT_sb, bc, fc1b_sb

            def recip_fast(out_ap, in_ap):
                from concourse.dve_ops import (
                    RECIP_APPROX_FAST_CONSTS,
                    RECIPROCAL_APPROX_FAST,
                )

                c = RECIP_APPROX_FAST_CONSTS
                nc.vector._custom_dve(
                    RECIPROCAL_APPROX_FAST, out=out_ap, in0=in_ap,
                    s0=c["s0"], s1=c["s1"], imm2=c["imm2"],
                )

            def attn_kb(pair, qc, pv, kb, start, stop):
                """One k-block: 2 scores MMs + exp + 2 PV MMs."""
                q0 = qc * 512
                duo = ps.tile([P, 1024], F32, tag="duo", name="duo")
                for hp in range(2):
                    nc.tensor.matmul(
                        duo[:, ts(hp, 512)],
                        xT[pair][ts(hp, DK), ts(kb, P)],
                        xT[pair][ts(hp, DK), ds(q0, 512)],
                        start=True, stop=True,
                    )
                e_t = e_pool.tile([P, 1024], BF16, tag="e", name="e_t")
                nc.scalar.activation(e_t[:], duo[:], AF.Exp, scale=1.0 / np.sqrt(DK))
                for hp in range(2):
                    h = 2 * pair + hp
                    nc.tensor.matmul(
                        pv[hp][0 : DK + 1, :],
                        xtok[kb][:, ds(h * HS, DK + 1)], e_t[:, ts(hp, 512)],
                        start=start, stop=stop,
                    )

            def attention_part1(l, pair, qc):
                """Own-half k-blocks; evicts partial O (bf16 SBUF) and the
                partial rowsum rows (SBUF scratch), freeing PSUM while the
                AllGather completes in the background."""
                pv = [ps.tile([P, 512], F32, tag="pv", name="pv") for _ in range(2)]
                for kb in range(NKB // 2):
                    attn_kb(pair, qc, pv, kb, kb == 0, kb == NKB // 2 - 1)
                g = qc * 4 + pair
                opart = op_pool.tile([P, 512], BF16, tag="opart", name="opart")
                for hp in range(2):
                    nc.vector.tensor_copy(
                        out=opart[ts(hp, DK), :], in_=pv[hp][0:DK, :]
                    )
                    nc.vector.tensor_copy(
                        out=sparts[g][0:1, ts(hp, 512)], in_=pv[hp][DK : DK + 1, :]
                    )
                return opart

            def part2_epilogue(pair, qc, pv, opart):
                """Combine peer-half PV with part1 + normalize into oT."""
                q0 = qc * 512
                g = qc * 4 + pair
                s2 = small.tile([1, 1024], F32, tag="s_sb", name="s2")
                for hp in range(2):
                    nc.vector.tensor_add(
                        s2[0:1, ts(hp, 512)], pv[hp][DK : DK + 1, :],
                        sparts[g][0:1, ts(hp, 512)],
                    )
                rf = small.tile([1, 1024], F32, tag="s_sb", name="rf")
                recip_fast(rf[:], s2[:])
                r2 = small.tile([1, 1024], F16, tag="r2", name="r2")
                nc.vector.tensor_copy(out=r2[:], in_=rf[:])
                bc_ps = ps.tile([P, 512], F32, tag="mm", name="bc_ps")
                for hp in range(2):
                    nc.tensor.matmul(
                        bc_ps[ts(hp, DK), :], ones1h[:], r2[0:1, ts(hp, 512)],
                        start=True, stop=True,
                    )
                for hp in range(2):
                    o_un = work.tile([P, 512], F32, tag="work", name="o_un")
                    nc.vector.tensor_add(
                        o_un[0:DK, :], pv[hp][0:DK, :], opart[ts(hp, DK), :]
                    )
                    dst = oT[pair][ts(hp, DK), ds(q0, 512)]
                    nc.vector.tensor_mul(dst, o_un[0:DK, :], bc_ps[ts(hp, DK), :])

            def ln_stats(src_tile):
                stats = small.tile([P, 6], F32, tag="stats")
                nc.vector.bn_stats(out=stats[:], in_=src_tile[:])
                mv = small.tile([P, 2], F32, tag="mv")
                nc.vector.bn_aggr(out=mv[:], in_=stats[:])
                return mv

            def ln_rstd_batch(mvs):
                n = len(mvs)
                vb = small.tile([P, 8], F32, tag="vb", name="vb")
                for i, mv in enumerate(mvs):
                    nc.vector.tensor_copy(out=vb[:, i : i + 1], in_=mv[:, 1:2])
                nc.scalar.activation(
                    out=vb[:, :n], in_=vb[:, :n], func=AF.Sqrt,
                    bias=epsT[:], scale=1.0,
                )
                nc.vector.reciprocal_approx_fast(out=vb[:, :n], in_=vb[:, :n])
                return vb

            def ln_apply(dst, src_tile, mv, rstd1, g_bc, b_bc):
                nc.vector.tensor_scalar(
                    out=dst[:], in0=src_tile[:],
                    scalar1=mv[:, 0:1], scalar2=rstd1,
                    op0=ALU.subtract, op1=ALU.mult,
                )
                if use_affine:
                    nc.vector.tensor_mul(dst[:], dst[:], g_bc[:])
                    nc.vector.tensor_add(dst[:], dst[:], b_bc[:])

            def wo_ln1(l, half, woT_sb, bc):
                ts_, mvs = [], []
                for q4 in range(4):
                    qt = half * 4 + q4
                    y_ps = ps.tile([P, D], F32, tag="mm", name="y_ps")
                    for oc in range(D // P):
                        nc.tensor.matmul(
                            y_ps[:], oT[oc][:, ts(qt, P)], woT_sb[oc][:],
                            start=(oc == 0), stop=(oc == D // P - 1),
                        )
                    t = work.tile([P, D], F32, tag="work")
                    nc.vector.tensor_add(t[:], y_ps[:], xres[qt][:])
                    if use_affine:
                        nc.vector.tensor_add(t[:], t[:], bc["wob"][:])
                    ts_.append(t)
                    mvs.append(ln_stats(t))
                vb = ln_rstd_batch(mvs)
                for q4 in range(4):
                    qt = half * 4 + q4
                    ln_apply(x1[qt], ts_[q4], mvs[q4], vb[:, q4 : q4 + 1],
                             bc.get("g1"), bc.get("b1"))
                    for ft in range(D // P):
                        tp = ps.tile([P, P], F16, tag="mm", name="tp")
                        nc.tensor.transpose(tp[:], x1[qt][:, ts(ft, P)], identh[:])
                        nc.vector.tensor_copy(out=x1T[ft][:, ts(qt, P)], in_=tp[:])

            def ffn_s1_step(l, qc, jc, facc01, hTs, w2s, fc1b_sb):
                """fc1(jc) + hT store + fc2 into q-tiles 0,1."""
                h_ps = ps.tile([P, 512], F32, tag="duo", name="h_ps")
                w1c = w1_pool.tile([P, D], F16, tag="w1c", name="w1c")
                nc.sync.dma_start(
                    w1c[:], w1T_d[l, jc].rearrange("p a j -> p (a j)")
                )
                for oc in range(D // P):
                    nc.tensor.matmul(
                        h_ps[:], w1c[:, ts(oc, P)],
                        x1T[oc][:, ds(qc * 512, 512)],
                        start=(oc == 0), stop=(oc == D // P - 1),
                    )
                hT = h_pool.tile([P, 512], F16, tag="hT", name="hT")
                if use_affine:
                    nc.vector.tensor_scalar_add(hT[:], h_ps[:], fc1b_sb[:, jc : jc + 1])
                else:
                    nc.vector.tensor_copy(out=hT[:], in_=h_ps[:])
                hTs.append(hT)
                w2c = w2_pool.tile([P, D], F16, tag="w2c", name="w2c")
                nc.sync.dma_start(w2c[:], w2T_d[l, jc])
                w2s.append(w2c)
                for q4 in range(2):
                    nc.tensor.matmul(
                        facc01[:, ts(q4, 512)], hTs[jc][:, ts(q4, P)], w2s[jc][:],
                        start=(jc == 0), stop=(jc == DFF // P - 1),
                    )

            def ffn_s2_step(l, qc, jc, facc23, hTs, w2s):
                for q4 in range(2, 4):
                    nc.tensor.matmul(
                        facc23[:, ts(q4 - 2, 512)], hTs[jc][:, ts(q4, P)], w2s[jc][:],
                        start=(jc == 0), stop=(jc == DFF // P - 1),
                    )

            def ffn_t2(qc, q4, facc_half, bc):
                qt = qc * 4 + q4
                t2 = work.tile([P, D], F32, tag="work")
                nc.vector.tensor_add(t2[:], facc_half[:, ts(q4 % 2, 512)], x1[qt][:])
                if use_affine:
                    nc.vector.tensor_add(t2[:], t2[:], bc["fc2b"][:])
                return t2, ln_stats(t2)

            def ffn_finish(l, qc, t2s, mvs2, bc):
                vb2 = ln_rstd_batch(mvs2)
                for q4 in range(4):
                    qt = qc * 4 + q4
                    ln_apply(xres[qt], t2s[q4], mvs2[q4],
                             vb2[:, q4 : q4 + 1], bc.get("g2"), bc.get("b2"))
                    if l == n_layers - 1:
                        nc.sync.dma_start(out_d[ts(qt, P), :], xres[qt][:])

            def rebuild_qt(l, i):
                """xtok tile i and xT col-block i from xres[i] (= new x)."""
                src3 = xres[i][:].rearrange("p (h k) -> p h k", k=DK)
                dst3 = xtok[i][:].rearrange("p (h k) -> p h k", k=HS)
                nc.vector.tensor_copy(out=dst3[:, :, 0:DK], in_=src3)
                nc.sync.dma_start(agt_ins[l][i], xtok[i][:])
                for ft in range(D // P):
                    tp = ps.tile([P, P], F16, tag="mm", name="tp")
                    nc.tensor.transpose(tp[:], xres[i][:, ts(ft, P)], identh[:])
                    nc.vector.tensor_copy(out=xT[ft][:, ts(i, P)], in_=tp[:])

            def fetch_peer(l):
                """xT cols 1024:2048 / xtok tiles 8..15 straight from the
                AllGather output — no rebuild compute on the receiver."""
                agf_flat = agf_outs[l][:].rearrange("r f p q -> (r f p) q")
                for ft in range(D // P):
                    nc.gpsimd.indirect_dma_start(
                        out=xT[ft][:, QH:S],
                        out_offset=None,
                        in_=agf_flat,
                        in_offset=bass.IndirectOffsetOnAxis(
                            ap=idxf_sb[:, ft : ft + 1], axis=0
                        ),
                    )
                agt_flat = agt_outs[l][:].rearrange("r i p w -> (r i p) w")
                for i in range(NQT):
                    nc.gpsimd.indirect_dma_start(
                        out=xtok[NQT + i][:],
                        out_offset=None,
                        in_=agt_flat,
                        in_offset=bass.IndirectOffsetOnAxis(
                            ap=idxt_sb[:, i : i + 1], axis=0
                        ),
                    )

            def emit_part1(l):
                oparts = {}
                for qc in range(NQC):
                    for pair in range(4):
                        oparts[(qc, pair)] = attention_part1(l, pair, qc)
                return oparts

            def part2_alone(l, qc, oparts):
                for pair in range(4):
                    pv = [ps.tile([P, 512], F32, tag="pv", name="pv") for _ in range(2)]
                    for kb_i in range(NKB // 2):
                        attn_kb(pair, qc, pv, NKB // 2 + kb_i,
                                kb_i == 0, kb_i == NKB // 2 - 1)
                    part2_epilogue(pair, qc, pv, oparts[(qc, pair)])

            def ffn_sweeps(l, qc, bc, fc1b_sb, attn=None, rebuild=False):
                """Two-sweep FFN for q-chunk qc. If `attn` is (qc_a, oparts),
                part2(qc_a) k-block steps interleave 2:1 with sweep-1 jc
                steps so softmax exps keep the scalar engine fed while the
                PE runs FFN matmuls. If `rebuild`, rebuild_qt(0..3) steps
                interleave instead (their xres came from the prior qc)."""
                facc01 = ps.tile([P, 1024], F32, tag="facc", name="facc01", bufs=1)
                hTs, w2s = [], []
                jc = 0
                if attn is not None:
                    qc_a, oparts = attn
                    for pair in range(4):
                        pv = [ps.tile([P, 512], F32, tag="pv", name="pv") for _ in range(2)]
                        for kb_i in range(NKB // 2):
                            attn_kb(pair, qc_a, pv, NKB // 2 + kb_i,
                                    kb_i == 0, kb_i == NKB // 2 - 1)
                            if kb_i % 2 == 1:
                                ffn_s1_step(l, qc, jc, facc01, hTs, w2s, fc1b_sb)
                                jc += 1
                        part2_epilogue(pair, qc_a, pv, oparts[(qc_a, pair)])
                while jc < DFF // P:
                    ffn_s1_step(l, qc, jc, facc01, hTs, w2s, fc1b_sb)
                    if rebuild and jc % 4 == 3 and l < n_layers - 1:
                        rebuild_qt(l, jc // 4)
                    jc += 1
                t2s, mvs2 = [], []
                for q4 in range(2):
                    t2, mv = ffn_t2(qc, q4, facc01, bc)
                    t2s.append(t2); mvs2.append(mv)
                facc23 = ps.tile([P, 1024], F32, tag="facc", name="facc23", bufs=1)
                for jc in range(DFF // P):
                    ffn_s2_step(l, qc, jc, facc23, hTs, w2s)
                for q4 in range(2, 4):
                    t2, mv = ffn_t2(qc, q4, facc23, bc)
                    t2s.append(t2); mvs2.append(mv)
                ffn_finish(l, qc, t2s, mvs2, bc)

            def ffn_plain(l, qc, bc, fc1b_sb):
                facc = [ps.tile([P, 1024], F32, tag="duo", name="facc") for _ in range(2)]
                for jc in range(DFF // P):
                    h_ps = ps.tile([P, 512], F32, tag="pv", name="h_ps")
                    w1c = w1_pool.tile([P, D], F16, tag="w1c", name="w1c")
                    nc.sync.dma_start(
                        w1c[:], w1T_d[l, jc].rearrange("p a j -> p (a j)")
                    )
                    for oc in range(D // P):
                        nc.tensor.matmul(
                            h_ps[:], w1c[:, ts(oc, P)],
                            x1T[oc][:, ds(qc * 512, 512)],
                            start=(oc == 0), stop=(oc == D // P - 1),
                        )
                    hT = h_pool.tile([P, 512], F16, tag="hT", name="hT")
                    if use_affine:
                        nc.vector.tensor_scalar_add(hT[:], h_ps[:], fc1b_sb[:, jc : jc + 1])
                    else:
                        nc.vector.tensor_copy(out=hT[:], in_=h_ps[:])
                    w2c = w2_pool.tile([P, D], F16, tag="w2c", name="w2c")
                    nc.sync.dma_start(w2c[:], w2T_d[l, jc])
                    for q4 in range(4):
                        nc.tensor.matmul(
                            facc[q4 // 2][:, ts(q4 % 2, 512)], hT[:, ts(q4, P)], w2c[:],
                            start=(jc == 0), stop=(jc == DFF // P - 1),
                        )
                t2s, mvs2 = [], []
                for q4 in range(4):
                    t2, mv = ffn_t2(qc, q4, facc[q4 // 2], bc)
                    t2s.append(t2); mvs2.append(mv)
                ffn_finish(l, qc, t2s, mvs2, bc)

            # ---- the stack ----
            oparts = None
            for l in range(n_layers):
                woT_sb, bc, fc1b_sb = load_layer_weights(l)
                if l == 0:
                    oparts = emit_part1(0)
                part2_alone(l, 0, oparts)
                part2_alone(l, 1, oparts)
                wo_ln1(l, 0, woT_sb, bc)
                wo_ln1(l, 1, woT_sb, bc)
                ffn_plain(l, 0, bc, fc1b_sb)
                ffn_plain(l, 1, bc, fc1b_sb)
                if l < n_layers - 1:
                    for i in range(NQT):
                        rebuild_qt(l, i)
                    for ft in range(D // P):
                        nc.sync.dma_start(agf_ins[l][ft], xT[ft][:, 0:QH])
                    nc.gpsimd.collective_compute(
                        "AllGather", ALU.bypass,
                        ins=[agf_ins[l][:].opt()],
                        outs=[agf_outs[l][:].opt()],
                        replica_groups=[[0, 1], [2, 3], [4, 5], [6, 7]],
                    )
                    nc.gpsimd.collective_compute(
                        "AllGather", ALU.bypass,
                        ins=[agt_ins[l][:].opt()],
                        outs=[agt_outs[l][:].opt()],
                        replica_groups=[[0, 1], [2, 3], [4, 5], [6, 7]],
                    )
                    oparts = emit_part1(l + 1)
                    fetch_peer(l)

    nc.compile()
    return nc


# ---- host side ----

_cache = {}


def _get_nc(n_layers=N_LAYERS, use_affine=False):
    key = (n_layers, use_affine)
    if key not in _cache:
        _cache[key] = build(n_layers, use_affine)
    return _cache[key]


def _trivial_affine(inputs, n_layers):
    return (
        not np.any(np.asarray(inputs["wo_b"], np.float32)[:n_layers])
        and not np.any(np.asarray(inputs["fc1_b"], np.float32)[:n_layers])
        and not np.any(np.asarray(inputs["fc2_b"], np.float32)[:n_layers])
        and not np.any(np.asarray(inputs["ln1_b"], np.float32)[:n_layers])
        and not np.any(np.asarray(inputs["ln2_b"], np.float32)[:n_layers])
        and np.all(np.asarray(inputs["ln1_g"], np.float32)[:n_layers] == 1.0)
        and np.all(np.asarray(inputs["ln2_g"], np.float32)[:n_layers] == 1.0)
    )


def make_in_maps(inputs, n_layers=N_LAYERS):
    f16 = ml_dtypes.float16 if hasattr(ml_dtypes, "float16") else np.float16
    x = np.asarray(inputs["x"], dtype=np.float32)
    woT = np.asarray(inputs["wo_w"], np.float32)[:n_layers].transpose(0, 2, 1)
    woT = np.ascontiguousarray(
        woT.reshape(n_layers, D // P, P, D)).astype(np.float16)
    w1T = np.asarray(inputs["fc1_w"], np.float32)[:n_layers].transpose(0, 2, 1)
    # [l, d, j] -> [l, jc, p(d-part within oc? no: p is d%?)]
    # desired tile[l, jc, p, oc, jj] = w1T[l, oc*128+p, jc*128+jj]
    w1T = w1T.reshape(n_layers, D // P, P, DFF // P, P).transpose(0, 3, 2, 1, 4)
    w1T = np.ascontiguousarray(w1T).astype(np.float16)
    w2T = np.asarray(inputs["fc2_w"], np.float32)[:n_layers].transpose(0, 2, 1)
    w2T = np.ascontiguousarray(
        w2T.reshape(n_layers, DFF // P, P, D)).astype(np.float16)
    common = {
        "woT": woT, "w1T": w1T, "w2T": w2T,
        "wob": np.ascontiguousarray(np.asarray(inputs["wo_b"], np.float32)[:n_layers]),
        "fc1b": np.ascontiguousarray(np.asarray(inputs["fc1_b"], np.float32)[:n_layers]),
        "fc2b": np.ascontiguousarray(np.asarray(inputs["fc2_b"], np.float32)[:n_layers]),
        "g1": np.ascontiguousarray(np.asarray(inputs["ln1_g"], np.float32)[:n_layers]),
        "b1": np.ascontiguousarray(np.asarray(inputs["ln1_b"], np.float32)[:n_layers]),
        "g2": np.ascontiguousarray(np.asarray(inputs["ln2_g"], np.float32)[:n_layers]),
        "b2": np.ascontiguousarray(np.asarray(inputs["ln2_b"], np.float32)[:n_layers]),
    }
    in_maps = []
    for c in range(NC):
        b, half = c // 2, c % 2
        own = x[b, half * QH : (half + 1) * QH]        # [QH, D]
        peer = x[b, (1 - half) * QH : (2 - half) * QH]
        local = np.concatenate([own, peer], axis=0)     # [S, D] core-relative
        x_feat = np.ascontiguousarray(local.T).astype(np.float16)  # [D, S]
        xt = np.zeros((S, H, HS), np.float32)
        xt[:, :, :DK] = local.reshape(S, H, DK)
        xt[:, :, DK] = 1.0
        x_tok = xt.reshape(S, XTW).astype(ml_dtypes.bfloat16)
        # peer slot within the 2-rank AllGather group
        pr = 1 - half
        p_ar = np.arange(P, dtype=np.int32)
        idxf = np.ascontiguousarray(
            (pr * (D // P) * P + np.arange(D // P, dtype=np.int32)[None, :] * P
             + p_ar[:, None]))
        idxt = np.ascontiguousarray(
            (pr * NQT * P + np.arange(NQT, dtype=np.int32)[None, :] * P
             + p_ar[:, None]))
        m = dict(common)
        m.update({
            "x_feat": x_feat, "x_tok": x_tok,
            "x_own": np.ascontiguousarray(own).astype(np.float16),
            "idxf": idxf, "idxt": idxt,
        })
        in_maps.append(m)
    return in_maps


def assemble_output(results):
    out = np.empty((B, S, D), np.float32)
    for c in range(NC):
        b, half = c // 2, c % 2
        out[b, half * QH : (half + 1) * QH] = np.asarray(
            results[c]["out"], dtype=np.float32)
    return out


def kernel(**inputs):
    from concourse.bass_utils import run_bass_kernel_spmd

    use_affine = not _trivial_affine(inputs, N_LAYERS)
    nc = _get_nc(N_LAYERS, use_affine)
    in_maps = make_in_maps(inputs)
    res = run_bass_kernel_spmd(nc, in_maps, core_ids=list(range(NC)))
    return assemble_output(res.results)


# revision 19
# speedup vs baseline: 1.1659x; 1.0139x over previous
"""Trainium2 Bass kernel for nn_Encoder_73778948211333.

6-layer transformer encoder (no qkv projections: q=k=v=head slices of x),
B=4, S=2048, D=512, H=8 heads, DFF=2048, fp32, no activation between fc1/fc2.

Sharding: 8 cores = (batch, sequence-half). Each core owns 1024 query rows of
one batch: computes attention for its rows (k-major scores -> exp -> PV with a
fused ones-column rowsum), wo + LN1 + FFN + LN2 for its rows, then a PAIRWISE
AllGather ([0,1],[2,3],...) exchanges updated halves between layers. The
payload carries the already-transposed feature-major tiles (xT, fp16) and the
token-major value tiles (xtok, bf16) so the receiver does zero rebuild
compute — peer tiles land via indirect row-gather DMAs.

All matmuls run in fp16/bf16 (full PE rate with fast-weight-load; the fp32
"HIGH" mode the previous version used streams at less than half rate on real
silicon). Softmax skips max-subtraction: scores are bounded since every layer
output is layer-normalized; exp outputs bf16 (large dynamic range).
LN gains/biases that are exactly 1/0 in the inputs are skipped at build time
(checked host-side; a general build is used otherwise).
"""

import sys

sys.path.insert(0, "/opt/trn_rl_repo")
sys.path.insert(0, "/root/.axon_site")

import numpy as np
import ml_dtypes

import concourse.bass as bass
import concourse.tile as tile
from concourse import bacc, mybir
from concourse.bass import ds, ts
from concourse.masks import make_identity

# ---- problem constants (hardcoded per spec) ----
B, S, D = 4, 2048, 512
H, DK = 8, 64
DFF = 4 * D
N_LAYERS = 6
EPS = 1e-8
P = 128
NC = 8
QH = S // 2          # 1024 rows per core
NKB = S // P         # 16 k-blocks
NQT = QH // P        # 8 q-tiles per core
NQC = QH // 512      # 2 q-chunks of 512
XTW = H * (DK + 1)   # 520: token-major row width incl. ones columns
HS = DK + 1          # per-head stride in xtok

F32 = mybir.dt.float32
F16 = mybir.dt.float16
BF16 = mybir.dt.bfloat16
I32 = mybir.dt.int32
AF = mybir.ActivationFunctionType
ALU = mybir.AluOpType


def build(n_layers=N_LAYERS, use_affine=False):
    nc = bacc.Bacc("TRN2", target_bir_lowering=False, debug=False, num_devices=NC)

    # ---- I/O ----
    x_feat = nc.declare_dram_parameter("x_feat", [D, S], F16, isOutput=False)
    x_tok = nc.declare_dram_parameter("x_tok", [S, XTW], BF16, isOutput=False)
    x_own = nc.declare_dram_parameter("x_own", [QH, D], F16, isOutput=False)
    idxf_in = nc.declare_dram_parameter("idxf", [P, D // P], I32, isOutput=False)
    idxt_in = nc.declare_dram_parameter("idxt", [P, NQT], I32, isOutput=False)
    # woT blocked [l, oc, 128, D]: rhs tiles for the wo matmul
    woT_d = nc.declare_dram_parameter("woT", [n_layers, D // P, P, D], F16, isOutput=False)
    # w1T blocked [l, jc, 128(p=d), 4(oc), 128(j)]: one [128,512] tile per jc
    w1T_d = nc.declare_dram_parameter(
        "w1T", [n_layers, DFF // P, P, D // P, P], F16, isOutput=False)
    # w2T blocked [l, jc, 128(p=j), D(o)]
    w2T_d = nc.declare_dram_parameter("w2T", [n_layers, DFF // P, P, D], F16, isOutput=False)
    wob_d = nc.declare_dram_parameter("wob", [n_layers, D], F32, isOutput=False)
    fc1b_d = nc.declare_dram_parameter("fc1b", [n_layers, DFF], F32, isOutput=False)
    fc2b_d = nc.declare_dram_parameter("fc2b", [n_layers, D], F32, isOutput=False)
    g1_d = nc.declare_dram_parameter("g1", [n_layers, D], F32, isOutput=False)
    b1_d = nc.declare_dram_parameter("b1", [n_layers, D], F32, isOutput=False)
    g2_d = nc.declare_dram_parameter("g2", [n_layers, D], F32, isOutput=False)
    b2_d = nc.declare_dram_parameter("b2", [n_layers, D], F32, isOutput=False)
    out_d = nc.declare_dram_parameter("out", [QH, D], F16, isOutput=True)

    # collective staging: xT own half [4, 128, 1024] f16 + xtok own [8, 128, 520] bf16
    agf_ins = [nc.dram_tensor(f"agf_in{l}", [D // P, P, QH], F16) for l in range(n_layers - 1)]
    agt_ins = [nc.dram_tensor(f"agt_in{l}", [NQT, P, XTW], BF16) for l in range(n_layers - 1)]
    agf_outs = [
        nc.dram_tensor(f"agf_out{l}", [2, D // P, P, QH], F16)
        for l in range(n_layers - 1)
    ]
    agt_outs = [
        nc.dram_tensor(f"agt_out{l}", [2, NQT, P, XTW], BF16)
        for l in range(n_layers - 1)
    ]

    with nc.allow_low_precision(reason="deliberate f16/bf16 compute"), tile.TileContext(nc) as tc:
        from contextlib import ExitStack

        with ExitStack() as ctx:
            persist = ctx.enter_context(tc.tile_pool(name="persist", bufs=1))
            wo_pool = ctx.enter_context(tc.tile_pool(name="wo_pool", bufs=5))
            w1_pool = ctx.enter_context(tc.tile_pool(name="w1_pool", bufs=4))
            w2_pool = ctx.enter_context(tc.tile_pool(name="w2_pool", bufs=16))
            b_pool = ctx.enter_context(tc.tile_pool(name="b_pool", bufs=7))
            fb_pool = ctx.enter_context(tc.tile_pool(name="fb_pool", bufs=2))
            e_pool = ctx.enter_context(tc.tile_pool(name="e_pool", bufs=4))
            h_pool = ctx.enter_context(tc.tile_pool(name="h_pool", bufs=16))
            work = ctx.enter_context(tc.tile_pool(name="work", bufs=4))
            small = ctx.enter_context(tc.tile_pool(name="small", bufs=6))
            op_pool = ctx.enter_context(tc.tile_pool(name="op_pool", bufs=8))
            # PSUM budget (16KB/partition): duo 2x4KB + facc 1x4KB + pv 2x2KB
            ps = ctx.enter_context(tc.tile_pool(name="ps", bufs=2, space="PSUM"))

            # ---- persistent state ----
            xT = [persist.tile([P, S], F16, tag=f"xT{i}", name=f"xT{i}") for i in range(D // P)]
            xtok = [persist.tile([P, XTW], BF16, tag=f"xtok{i}", name=f"xtok{i}") for i in range(NKB)]
            xres = [persist.tile([P, D], F16, tag=f"xres{i}", name=f"xres{i}") for i in range(NQT)]
            x1 = [persist.tile([P, D], F16, tag=f"x1_{i}", name=f"x1_{i}") for i in range(NQT)]
            x1T = [persist.tile([P, QH], F16, tag=f"x1T{i}", name=f"x1T{i}") for i in range(D // P)]
            oT = [persist.tile([P, QH], F16, tag=f"oT{i}", name=f"oT{i}") for i in range(D // P)]
            identh = persist.tile([P, P], F16, tag="identh")
            identf = persist.tile([P, P], F32, tag="identf")
            ones1h = persist.tile([1, DK], F16, tag="ones1h")
            epsT = persist.tile([P, 1], F32, tag="epsT")
            # partial softmax denominators: one [1, 1024] tile per (qc, pair)
            # group (hp0 in cols 0:512, hp1 in 512:1024; partition 0 only —
            # partition bases must be 32-aligned so no [2, ...] row tiles)
            sparts = [persist.tile([1, 1024], F32, tag=f"sp{g}", name=f"sp{g}")
                      for g in range(NQC * 4)]
            idxf_sb = persist.tile([P, D // P], I32, tag="idxf_sb")
            idxt_sb = persist.tile([P, NQT], I32, tag="idxt_sb")

            make_identity(nc, identf[:])
            nc.vector.tensor_copy(out=identh[:], in_=identf[:])
            nc.vector.memset(ones1h[:], 1.0)
            nc.vector.memset(epsT[:], EPS)
            nc.sync.dma_start(idxf_sb[:], idxf_in[:])
            nc.sync.dma_start(idxt_sb[:], idxt_in[:])

            # ---- initial loads ----
            for i in range(D // P):
                nc.sync.dma_start(xT[i][:], x_feat[ts(i, P), :])
            for i in range(NKB):
                nc.sync.dma_start(xtok[i][:], x_tok[ts(i, P), :])
            for i in range(NQT):
                nc.sync.dma_start(xres[i][:], x_own[ts(i, P), :])

            def load_layer_weights(l):
                woT_sb = []
                for oc in range(D // P):
                    t = wo_pool.tile([P, D], F16, tag="woT_sb", name="woT_sb")
                    nc.sync.dma_start(t[:], woT_d[l, oc])
                    woT_sb.append(t)
                bc = {}
                if use_affine:
                    for name, dram in (
                        ("wob", wob_d), ("fc2b", fc2b_d),
                        ("g1", g1_d), ("b1", b1_d), ("g2", g2_d), ("b2", b2_d),
                    ):
                        t = b_pool.tile([P, D], F32, tag="bc", name="bc")
                        nc.sync.dma_start(t[:], dram[l, None, :].to_broadcast((P, D)))
                        bc[name] = t
                    fc1b_sb = fb_pool.tile([P, DFF // P], F32, tag="fc1b_sb")
                    nc.sync.dma_start(
                        fc1b_sb[:], fc1b_d[l, :].rearrange("(a p) -> p a", p=P)
                    )
                else:
                    fc1b_sb = None
                return woT_sb, bc, fc1b_sb

            def recip_fast(out_ap, in_ap):
                from concourse.dve_ops import (
                    RECIP_APPROX_FAST_CONSTS,
                    RECIPROCAL_APPROX_FAST,
                )

                c = RECIP_APPROX_FAST_CONSTS
                nc.vector._custom_dve(
                    RECIPROCAL_APPROX_FAST, out=out_ap, in0=in_ap,
                    s0=c["s0"], s1=c["s1"], imm2=c["imm2"],
                )

            def attn_kb(pair, qc, pv, kb, start, stop):
                """One k-block: 2 scores MMs + exp + 2 PV MMs."""
                q0 = qc * 512
                duo = ps.tile([P, 1024], F32, tag="duo", name="duo")
                for hp in range(2):
                    nc.tensor.matmul(
                        duo[:, ts(hp, 512)],
                        xT[pair][ts(hp, DK), ts(kb, P)],
                        xT[pair][ts(hp, DK), ds(q0, 512)],
                        start=True, stop=True,
                    )
                e_t = e_pool.tile([P, 1024], BF16, tag="e", name="e_t")
                nc.scalar.activation(e_t[:], duo[:], AF.Exp, scale=1.0 / np.sqrt(DK))
                for hp in range(2):
                    h = 2 * pair + hp
                    nc.tensor.matmul(
                        pv[hp][0 : DK + 1, :],
                        xtok[kb][:, ds(h * HS, DK + 1)], e_t[:, ts(hp, 512)],
                        start=start, stop=stop,
                    )

            def attention_part1(l, pair, qc):
                """Own-half k-blocks; evicts partial O (bf16 SBUF) and the
                partial rowsum rows (SBUF scratch), freeing PSUM while the
                AllGather completes in the background."""
                pv = [ps.tile([P, 512], F32, tag="pv", name="pv") for _ in range(2)]
                for kb in range(NKB // 2):
                    attn_kb(pair, qc, pv, kb, kb == 0, kb == NKB // 2 - 1)
                g = qc * 4 + pair
                opart = op_pool.tile([P, 512], BF16, tag="opart", name="opart")
                for hp in range(2):
                    nc.vector.tensor_copy(
                        out=opart[ts(hp, DK), :], in_=pv[hp][0:DK, :]
                    )
                    nc.vector.tensor_copy(
                        out=sparts[g][0:1, ts(hp, 512)], in_=pv[hp][DK : DK + 1, :]
                    )
                return opart

            def part2_epilogue(pair, qc, pv, opart):
                """Combine peer-half PV with part1 + normalize into oT."""
                q0 = qc * 512
                g = qc * 4 + pair
                s2 = small.tile([1, 1024], F32, tag="s_sb", name="s2")
                for hp in range(2):
                    nc.vector.tensor_add(
                        s2[0:1, ts(hp, 512)], pv[hp][DK : DK + 1, :],
                        sparts[g][0:1, ts(hp, 512)],
                    )
                rf = small.tile([1, 1024], F32, tag="s_sb", name="rf")
                recip_fast(rf[:], s2[:])
                r2 = small.tile([1, 1024], F16, tag="r2", name="r2")
                nc.vector.tensor_copy(out=r2[:], in_=rf[:])
                bc_ps = ps.tile([P, 512], F32, tag="mm", name="bc_ps")
                for hp in range(2):
                    nc.tensor.matmul(
                        bc_ps[ts(hp, DK), :], ones1h[:], r2[0:1, ts(hp, 512)],
                        start=True, stop=True,
                    )
                for hp in range(2):
                    o_un = work.tile([P, 512], F32, tag="work", name="o_un")
                    nc.vector.tensor_add(
                        o_un[0:DK, :], pv[hp][0:DK, :], opart[ts(hp, DK), :]
                    )
                    dst = oT[pair][ts(hp, DK), ds(q0, 512)]
                    nc.vector.tensor_mul(dst, o_un[0:DK, :], bc_ps[ts(hp, DK), :])

            def ln_stats(src_tile):
                stats = small.tile([P, 6], F32, tag="stats")
                nc.vector.bn_stats(out=stats[:], in_=src_tile[:])
                mv = small.tile([P, 2], F32, tag="mv")
                nc.vector.bn_aggr(out=mv[:], in_=stats[:])
                return mv

            def ln_rstd_batch(mvs):
                n = len(mvs)
                vb = small.tile([P, 8], F32, tag="vb", name="vb")
                for i, mv in enumerate(mvs):
                    nc.vector.tensor_copy(out=vb[:, i : i + 1], in_=mv[:, 1:2])
                nc.scalar.activation(
                    out=vb[:, :n], in_=vb[:, :n], func=AF.Sqrt,
                    bias=epsT[:], scale=1.0,
                )
                nc.vector.reciprocal_approx_fast(out=vb[:, :n], in_=vb[:, :n])
                return vb

            def ln_apply(dst, src_tile, mv, rstd1, g_bc, b_bc):
                nc.vector.tensor_scalar(
                    out=dst[:], in0=src_tile[:],
                    scalar1=mv[:, 0:1], scalar2=rstd1,
                    op0=ALU.subtract, op1=ALU.mult,
                )
                if use_affine:
                    nc.vector.tensor_mul(dst[:], dst[:], g_bc[:])
                    nc.vector.tensor_add(dst[:], dst[:], b_bc[:])

            def wo_ln1(l, half, woT_sb, bc):
                ts_, mvs = [], []
                for q4 in range(4):
                    qt = half * 4 + q4
                    y_ps = ps.tile([P, D], F32, tag="mm", name="y_ps")
                    for oc in range(D // P):
                        nc.tensor.matmul(
                            y_ps[:], oT[oc][:, ts(qt, P)], woT_sb[oc][:],
                            start=(oc == 0), stop=(oc == D // P - 1),
                        )
                    t = work.tile([P, D], F32, tag="work")
                    nc.vector.tensor_add(t[:], y_ps[:], xres[qt][:])
                    if use_affine:
                        nc.vector.tensor_add(t[:], t[:], bc["wob"][:])
                    ts_.append(t)
                    mvs.append(ln_stats(t))
                vb = ln_rstd_batch(mvs)
                for q4 in range(4):
                    qt = half * 4 + q4
                    ln_apply(x1[qt], ts_[q4], mvs[q4], vb[:, q4 : q4 + 1],
                             bc.get("g1"), bc.get("b1"))
                    for ft in range(D // P):
                        tp = ps.tile([P, P], F16, tag="mm", name="tp")
                        nc.tensor.transpose(tp[:], x1[qt][:, ts(ft, P)], identh[:])
                        nc.vector.tensor_copy(out=x1T[ft][:, ts(qt, P)], in_=tp[:])

            def ffn_s1_step(l, qc, jc, facc01, hTs, w2s, fc1b_sb):
                """fc1(jc) + hT store + fc2 into q-tiles 0,1."""
                h_ps = ps.tile([P, 512], F32, tag="duo", name="h_ps")
                w1c = w1_pool.tile([P, D], F16, tag="w1c", name="w1c")
                nc.sync.dma_start(
                    w1c[:], w1T_d[l, jc].rearrange("p a j -> p (a j)")
                )
                for oc in range(D // P):
                    nc.tensor.matmul(
                        h_ps[:], w1c[:, ts(oc, P)],
                        x1T[oc][:, ds(qc * 512, 512)],
                        start=(oc == 0), stop=(oc == D // P - 1),
                    )
                hT = h_pool.tile([P, 512], F16, tag="hT", name="hT")
                if use_affine:
                    nc.vector.tensor_scalar_add(hT[:], h_ps[:], fc1b_sb[:, jc : jc + 1])
                else:
                    nc.vector.tensor_copy(out=hT[:], in_=h_ps[:])
                hTs.append(hT)
                w2c = w2_pool.tile([P, D], F16, tag="w2c", name="w2c")
                nc.sync.dma_start(w2c[:], w2T_d[l, jc])
                w2s.append(w2c)
                for q4 in range(2):
                    nc.tensor.matmul(
                        facc01[:, ts(q4, 512)], hTs[jc][:, ts(q4, P)], w2s[jc][:],
                        start=(jc == 0), stop=(jc == DFF // P - 1),
                    )

            def ffn_s2_step(l, qc, jc, facc23, hTs, w2s):
                for q4 in range(2, 4):
                    nc.tensor.matmul(
                        facc23[:, ts(q4 - 2, 512)], hTs[jc][:, ts(q4, P)], w2s[jc][:],
                        start=(jc == 0), stop=(jc == DFF // P - 1),
                    )

            def ffn_t2(qc, q4, facc_half, bc):
                qt = qc * 4 + q4
                t2 = work.tile([P, D], F32, tag="work")
                nc.vector.tensor_add(t2[:], facc_half[:, ts(q4 % 2, 512)], x1[qt][:])
                if use_affine:
                    nc.vector.tensor_add(t2[:], t2[:], bc["fc2b"][:])
                return t2, ln_stats(t2)

            def ffn_finish(l, qc, t2s, mvs2, bc):
                vb2 = ln_rstd_batch(mvs2)
                for q4 in range(4):
                    qt = qc * 4 + q4
                    ln_apply(xres[qt], t2s[q4], mvs2[q4],
                             vb2[:, q4 : q4 + 1], bc.get("g2"), bc.get("b2"))
                    if l == n_layers - 1:
                        nc.sync.dma_start(out_d[ts(qt, P), :], xres[qt][:])

            def rebuild_qt(l, i):
                """xtok tile i and xT col-block i from xres[i] (= new x)."""
                src3 = xres[i][:].rearrange("p (h k) -> p h k", k=DK)
                dst3 = xtok[i][:].rearrange("p (h k) -> p h k", k=HS)
                nc.vector.tensor_copy(out=dst3[:, :, 0:DK], in_=src3)
                nc.sync.dma_start(agt_ins[l][i], xtok[i][:])
                for ft in range(D // P):
                    tp = ps.tile([P, P], F16, tag="mm", name="tp")
                    nc.tensor.transpose(tp[:], xres[i][:, ts(ft, P)], identh[:])
                    nc.vector.tensor_copy(out=xT[ft][:, ts(i, P)], in_=tp[:])

            def fetch_peer(l):
                """xT cols 1024:2048 / xtok tiles 8..15 straight from the
                AllGather output — no rebuild compute on the receiver."""
                agf_flat = agf_outs[l][:].rearrange("r f p q -> (r f p) q")
                for ft in range(D // P):
                    nc.gpsimd.indirect_dma_start(
                        out=xT[ft][:, QH:S],
                        out_offset=None,
                        in_=agf_flat,
                        in_offset=bass.IndirectOffsetOnAxis(
                            ap=idxf_sb[:, ft : ft + 1], axis=0
                        ),
                    )
                agt_flat = agt_outs[l][:].rearrange("r i p w -> (r i p) w")
                for i in range(NQT):
                    nc.gpsimd.indirect_dma_start(
                        out=xtok[NQT + i][:],
                        out_offset=None,
                        in_=agt_flat,
                        in_offset=bass.IndirectOffsetOnAxis(
                            ap=idxt_sb[:, i : i + 1], axis=0
                        ),
                    )

            def emit_part1(l):
                oparts = {}
                for qc in range(NQC):
                    for pair in range(4):
                        oparts[(qc, pair)] = attention_part1(l, pair, qc)
                return oparts

            def part2_alone(l, qc, oparts):
                for pair in range(4):
                    pv = [ps.tile([P, 512], F32, tag="pv", name="pv") for _ in range(2)]
                    for kb_i in range(NKB // 2):
                        attn_kb(pair, qc, pv, NKB // 2 + kb_i,
                                kb_i == 0, kb_i == NKB // 2 - 1)
                    part2_epilogue(pair, qc, pv, oparts[(qc, pair)])

            def ffn_sweeps(l, qc, bc, fc1b_sb, attn=None, rebuild=False):
                """Two-sweep FFN for q-chunk qc. If `attn` is (qc_a, oparts),
                part2(qc_a) k-block steps interleave 2:1 with sweep-1 jc
                steps so softmax exps keep the scalar engine fed while the
                PE runs FFN matmuls. If `rebuild`, rebuild_qt(0..3) steps
                interleave instead (their xres came from the prior qc)."""
                facc01 = ps.tile([P, 1024], F32, tag="facc", name="facc01", bufs=1)
                hTs, w2s = [], []
                jc = 0
                if attn is not None:
                    qc_a, oparts = attn
                    for pair in range(4):
                        pv = [ps.tile([P, 512], F32, tag="pv", name="pv") for _ in range(2)]
                        for kb_i in range(NKB // 2):
                            attn_kb(pair, qc_a, pv, NKB // 2 + kb_i,
                                    kb_i == 0, kb_i == NKB // 2 - 1)
                            if kb_i % 2 == 1:
                                ffn_s1_step(l, qc, jc, facc01, hTs, w2s, fc1b_sb)
                                jc += 1
                        part2_epilogue(pair, qc_a, pv, oparts[(qc_a, pair)])
                while jc < DFF // P:
                    ffn_s1_step(l, qc, jc, facc01, hTs, w2s, fc1b_sb)
                    if rebuild and jc % 4 == 3 and l < n_layers - 1:
                        rebuild_qt(l, jc // 4)
                    jc += 1
                t2s, mvs2 = [], []
                for q4 in range(2):
                    t2, mv = ffn_t2(qc, q4, facc01, bc)
                    t2s.append(t2); mvs2.append(mv)
                facc23 = ps.tile([P, 1024], F32, tag="facc", name="facc23", bufs=1)
                for jc in range(DFF // P):
                    ffn_s2_step(l, qc, jc, facc23, hTs, w2s)
                for q4 in range(2, 4):
                    t2, mv = ffn_t2(qc, q4, facc23, bc)
                    t2s.append(t2); mvs2.append(mv)
                ffn_finish(l, qc, t2s, mvs2, bc)

            def ffn_plain(l, qc, bc, fc1b_sb):
                facc = [ps.tile([P, 1024], F32, tag="duo", name="facc") for _ in range(2)]
                for jc in range(DFF // P):
                    h_ps = ps.tile([P, 512], F32, tag="pv", name="h_ps")
                    w1c = w1_pool.tile([P, D], F16, tag="w1c", name="w1c")
                    nc.sync.dma_start(
                        w1c[:], w1T_d[l, jc].rearrange("p a j -> p (a j)")
                    )
                    for oc in range(D // P):
                        nc.tensor.matmul(
                            h_ps[:], w1c[:, ts(oc, P)],
                            x1T[oc][:, ds(qc * 512, 512)],
                            start=(oc == 0), stop=(oc == D // P - 1),
                        )
                    hT = h_pool.tile([P, 512], F16, tag="hT", name="hT")
                    if use_affine:
                        nc.vector.tensor_scalar_add(hT[:], h_ps[:], fc1b_sb[:, jc : jc + 1])
                    else:
                        nc.scalar.activation(hT[:], h_ps[:], AF.Copy)
                    w2c = w2_pool.tile([P, D], F16, tag="w2c", name="w2c")
                    nc.sync.dma_start(w2c[:], w2T_d[l, jc])
                    for q4 in range(4):
                        nc.tensor.matmul(
                            facc[q4 // 2][:, ts(q4 % 2, 512)], hT[:, ts(q4, P)], w2c[:],
                            start=(jc == 0), stop=(jc == DFF // P - 1),
                        )
                t2s, mvs2 = [], []
                for q4 in range(4):
                    t2, mv = ffn_t2(qc, q4, facc[q4 // 2], bc)
                    t2s.append(t2); mvs2.append(mv)
                ffn_finish(l, qc, t2s, mvs2, bc)

            # ---- the stack ----
            oparts = None
            for l in range(n_layers):
                woT_sb, bc, fc1b_sb = load_layer_weights(l)
                if l == 0:
                    oparts = emit_part1(0)
                part2_alone(l, 0, oparts)
                part2_alone(l, 1, oparts)
                wo_ln1(l, 0, woT_sb, bc)
                wo_ln1(l, 1, woT_sb, bc)
                ffn_plain(l, 0, bc, fc1b_sb)
                ffn_plain(l, 1, bc, fc1b_sb)
                if l < n_layers - 1:
                    for i in range(NQT):
                        rebuild_qt(l, i)
                    for ft in range(D // P):
                        nc.sync.dma_start(agf_ins[l][ft], xT[ft][:, 0:QH])
                    nc.gpsimd.collective_compute(
                        "AllGather", ALU.bypass,
                        ins=[agf_ins[l][:].opt()],
                        outs=[agf_outs[l][:].opt()],
                        replica_groups=[[0, 1], [2, 3], [4, 5], [6, 7]],
                    )
                    nc.gpsimd.collective_compute(
                        "AllGather", ALU.bypass,
                        ins=[agt_ins[l][:].opt()],
                        outs=[agt_outs[l][:].opt()],
                        replica_groups=[[0, 1], [2, 3], [4, 5], [6, 7]],
                    )
                    oparts = emit_part1(l + 1)
                    fetch_peer(l)

    nc.compile()
    return nc


# ---- host side ----

_cache = {}


def _get_nc(n_layers=N_LAYERS, use_affine=False):
    key = (n_layers, use_affine)
    if key not in _cache:
        _cache[key] = build(n_layers, use_affine)
    return _cache[key]


def _trivial_affine(inputs, n_layers):
    return (
        not np.any(np.asarray(inputs["wo_b"], np.float32)[:n_layers])
        and not np.any(np.asarray(inputs["fc1_b"], np.float32)[:n_layers])
        and not np.any(np.asarray(inputs["fc2_b"], np.float32)[:n_layers])
        and not np.any(np.asarray(inputs["ln1_b"], np.float32)[:n_layers])
        and not np.any(np.asarray(inputs["ln2_b"], np.float32)[:n_layers])
        and np.all(np.asarray(inputs["ln1_g"], np.float32)[:n_layers] == 1.0)
        and np.all(np.asarray(inputs["ln2_g"], np.float32)[:n_layers] == 1.0)
    )


def make_in_maps(inputs, n_layers=N_LAYERS):
    f16 = ml_dtypes.float16 if hasattr(ml_dtypes, "float16") else np.float16
    x = np.asarray(inputs["x"], dtype=np.float32)
    woT = np.asarray(inputs["wo_w"], np.float32)[:n_layers].transpose(0, 2, 1)
    woT = np.ascontiguousarray(
        woT.reshape(n_layers, D // P, P, D)).astype(np.float16)
    w1T = np.asarray(inputs["fc1_w"], np.float32)[:n_layers].transpose(0, 2, 1)
    # [l, d, j] -> [l, jc, p(d-part within oc? no: p is d%?)]
    # desired tile[l, jc, p, oc, jj] = w1T[l, oc*128+p, jc*128+jj]
    w1T = w1T.reshape(n_layers, D // P, P, DFF // P, P).transpose(0, 3, 2, 1, 4)
    w1T = np.ascontiguousarray(w1T).astype(np.float16)
    w2T = np.asarray(inputs["fc2_w"], np.float32)[:n_layers].transpose(0, 2, 1)
    w2T = np.ascontiguousarray(
        w2T.reshape(n_layers, DFF // P, P, D)).astype(np.float16)
    common = {
        "woT": woT, "w1T": w1T, "w2T": w2T,
        "wob": np.ascontiguousarray(np.asarray(inputs["wo_b"], np.float32)[:n_layers]),
        "fc1b": np.ascontiguousarray(np.asarray(inputs["fc1_b"], np.float32)[:n_layers]),
        "fc2b": np.ascontiguousarray(np.asarray(inputs["fc2_b"], np.float32)[:n_layers]),
        "g1": np.ascontiguousarray(np.asarray(inputs["ln1_g"], np.float32)[:n_layers]),
        "b1": np.ascontiguousarray(np.asarray(inputs["ln1_b"], np.float32)[:n_layers]),
        "g2": np.ascontiguousarray(np.asarray(inputs["ln2_g"], np.float32)[:n_layers]),
        "b2": np.ascontiguousarray(np.asarray(inputs["ln2_b"], np.float32)[:n_layers]),
    }
    in_maps = []
    for c in range(NC):
        b, half = c // 2, c % 2
        own = x[b, half * QH : (half + 1) * QH]        # [QH, D]
        peer = x[b, (1 - half) * QH : (2 - half) * QH]
        local = np.concatenate([own, peer], axis=0)     # [S, D] core-relative
        x_feat = np.ascontiguousarray(local.T).astype(np.float16)  # [D, S]
        xt = np.zeros((S, H, HS), np.float32)
        xt[:, :, :DK] = local.reshape(S, H, DK)
        xt[:, :, DK] = 1.0
        x_tok = xt.reshape(S, XTW).astype(ml_dtypes.bfloat16)
        # peer slot within the 2-rank AllGather group
        pr = 1 - half
        p_ar = np.arange(P, dtype=np.int32)
        idxf = np.ascontiguousarray(
            (pr * (D // P) * P + np.arange(D // P, dtype=np.int32)[None, :] * P
             + p_ar[:, None]))
        idxt = np.ascontiguousarray(
            (pr * NQT * P + np.arange(NQT, dtype=np.int32)[None, :] * P
             + p_ar[:, None]))
        m = dict(common)
        m.update({
            "x_feat": x_feat, "x_tok": x_tok,
            "x_own": np.ascontiguousarray(own).astype(np.float16),
            "idxf": idxf, "idxt": idxt,
        })
        in_maps.append(m)
    return in_maps


def assemble_output(results):
    out = np.empty((B, S, D), np.float32)
    for c in range(NC):
        b, half = c // 2, c % 2
        out[b, half * QH : (half + 1) * QH] = np.asarray(
            results[c]["out"], dtype=np.float32)
    return out


def kernel(**inputs):
    from concourse.bass_utils import run_bass_kernel_spmd

    use_affine = not _trivial_affine(inputs, N_LAYERS)
    nc = _get_nc(N_LAYERS, use_affine)
    in_maps = make_in_maps(inputs)
    res = run_bass_kernel_spmd(nc, in_maps, core_ids=list(range(NC)))
    return assemble_output(res.results)
